# revision 14
# baseline (speedup 1.0000x reference)
"""Trainium2 Bass kernel for the KalmanFilter linear recurrence.

  x = data - mean;  z0 = R @ x[0];  drive = inputs @ C.T
  z_{t+1} = A z_t + drive[t]   (T = 32768 steps, dim 512)
  result  = Z[1:] @ B.T + mean

Strategy (8 NeuronCores, sequence-parallel, no collectives):
  - ||A^k|| decays like 0.9^k (spectral radius 0.9), so the recurrence
    forgets its state after H=128 steps to ~1e-5 relative.
  - Each core owns 4096 contiguous steps, split into 256 chunks of S=16
    steps + K=8 extra "halo" chunks covering the preceding H=128 steps.
  - Phase A: batched zero-init scan over all 264 chunks (state tiles
    [512, 264], 15 matmul steps) -> per-chunk accumulated drives b_c.
  - Phase B: chunk-start states w_c = sum_{p=0}^{K-1} (A^16)^p b_{c-1-p}
    (banded combine; truncated at ||A^128|| ~ 4e-4 of a unit).
  - Phase C: re-scan the 256 real chunks from inits w_c; each step also
    applies the output projection B.T (+mean) and streams rows to DRAM.
  - z0 only affects output rows 0..H-1 (through A^n z0); that correction
    is added on the host, so the device never sees `data`/`R`.

  Wall time is dominated by the host<->device tunnel (~55MB/s), so the
  wire format is aggressively compressed: inputs ship as int8 with
  per-feature scales folded into C.T on the host; outputs ship as int8
  rows with a per-row abs-max scale (f32) packed into 4 extra int8
  columns of the same tensor. Matrix constants ship fp16, packed into
  one tensor uploaded to core 0, broadcast device-to-device, and cached
  on device across calls keyed by a content hash. Donated output zero
  buffers are created on device. Matmuls run fp16 with f32 PSUM
  accumulation. The jit executable is built once and cached.
"""
import hashlib
from concurrent.futures import ThreadPoolExecutor
import numpy as np
import jax
import jax.numpy as jnp
from jax.experimental.shard_map import shard_map
from jax.sharding import Mesh, NamedSharding, PartitionSpec as P

import concourse.bacc as bacc
import concourse.mybir as mybir
from concourse import tile
from concourse.bass2jax import (
    _bass_exec_p, install_neuronx_cc_hook, partition_id_tensor)

T = 32768
DZ = 512
DU = 256
NCORE = 8
TLOC = T // NCORE          # 4096
S = 16                     # steps per chunk
BCH = TLOC // S            # 256 chunks per core
H = 128                    # halo steps (forgetting horizon)
K = H // S                 # 8 banded taps (incl. identity)
NCH = BCH + K              # 264 chunks in phase A
ULEN = TLOC + H            # 4224 drive rows per core (multiple of 128)
OW = DZ + 4                # 516: int8 row + 4 bytes of f32 row scale
# packed constants: at(512) bt(512) ct(256) mn(128) mb(7*512)
KROWS = 512 + 512 + 256 + 128 + (K - 1) * 512   # 4992
MBOFF = 1408

f16 = mybir.dt.float16
f32 = mybir.dt.float32
i8 = mybir.dt.int8

_CACHE = {}


def _emit(nc):
    u_d = nc.dram_tensor("u", (2 * 128, ULEN), i8, kind="ExternalInput")
    kon_d = nc.dram_tensor("kon", (KROWS, DZ), f16, kind="ExternalInput")
    out_d = nc.dram_tensor("out", (TLOC, OW), i8, kind="ExternalOutput")

    with tile.TileContext(nc) as tc:
        with tc.tile_pool(name="const", bufs=1) as cpool, \
             tc.tile_pool(name="dt", bufs=1) as dpool, \
             tc.tile_pool(name="ut", bufs=1) as upool, \
             tc.tile_pool(name="mb", bufs=3) as mbpool, \
             tc.tile_pool(name="st", bufs=2) as stpool, \
             tc.tile_pool(name="ob", bufs=4) as opool, \
             tc.tile_pool(name="sc", bufs=8) as scpool, \
             tc.tile_pool(name="ps", bufs=8, space="PSUM") as pp:

            # ---- constant loads (packed rows of kon) ----
            at_sb = [cpool.tile([128, DZ], f16, tag=f"at{k}", name=f"at{k}") for k in range(4)]
            bt_sb = [cpool.tile([128, DZ], f16, tag=f"bt{k}", name=f"bt{k}") for k in range(4)]
            ct_sb = [cpool.tile([128, DZ], f16, tag=f"ct{k}", name=f"ct{k}") for k in range(2)]
            mn_sb = cpool.tile([128, DZ], f16, tag="mn")
            for k in range(4):
                nc.sync.dma_start(at_sb[k][:], kon_d[128 * k:128 * (k + 1), :])
            for k in range(4):
                nc.sync.dma_start(bt_sb[k][:], kon_d[512 + 128 * k:512 + 128 * (k + 1), :])
            for k in range(2):
                nc.sync.dma_start(ct_sb[k][:], kon_d[1024 + 128 * k:1024 + 128 * (k + 1), :])
            nc.sync.dma_start(mn_sb[:], kon_d[1280:1408, :])

            # u.T tiles (int8 on the wire, widened to fp16 for the PE)
            uq_sb = [upool.tile([128, ULEN], i8, tag=f"uq{k}", name=f"uq{k}") for k in range(2)]
            ut_sb = [upool.tile([128, ULEN], f16, tag=f"ut{k}", name=f"ut{k}") for k in range(2)]
            for k in range(2):
                nc.sync.dma_start(uq_sb[k][:], u_d[128 * k:128 * (k + 1), :])
            for k in range(2):
                nc.vector.tensor_copy(ut_sb[k][:], uq_sb[k][:])

            # drive rows (transposed): dt[m] holds drive.T[128m:128(m+1), :]
            dt_sb = [dpool.tile([128, ULEN], f16, tag=f"dt{m}", name=f"dt{m}") for m in range(4)]
            for nb in range((ULEN + 511) // 512):
                nb0 = nb * 512
                w = min(512, ULEN - nb0)
                for m in range(4):
                    psd = pp.tile([128, 512], f32, tag="ps")
                    for kk in range(2):
                        nc.tensor.matmul(
                            psd[:, :w],
                            ct_sb[kk][:, 128 * m:128 * (m + 1)],
                            ut_sb[kk][:, nb0:nb0 + w],
                            start=(kk == 0), stop=(kk == 1))
                    nc.any.tensor_copy(dt_sb[m][:, nb0:nb0 + w], psd[:, :w])

            # ---- phase A: zero-init scan over NCH chunks ----
            bmat = [cpool.tile([128, NCH], f16, tag=f"bm{m}", name=f"bm{m}") for m in range(4)]
            st_prev = []
            for m in range(4):
                t0 = stpool.tile([128, NCH], f16, tag=f"st{m}", name=f"st0_{m}")
                nc.vector.tensor_copy(t0[:], dt_sb[m][:, 0:16 * (NCH - 1) + 1:16])
                st_prev.append(t0)
            for k in range(1, S):
                psl = [pp.tile([128, NCH], f32, tag="ps", name=f"psA{k}_{_m}") for _m in range(4)]
                for m in range(4):
                    for kk in range(4):
                        nc.tensor.matmul(
                            psl[m][:],
                            at_sb[kk][:, 128 * m:128 * (m + 1)],
                            st_prev[kk][:],
                            start=(kk == 0), stop=(kk == 3))
                st_new = []
                for m in range(4):
                    dst = (bmat[m] if k == S - 1 else
                           stpool.tile([128, NCH], f16, tag=f"st{m}", name=f"stA{k}_{m}"))
                    nc.vector.tensor_tensor(
                        dst[:], psl[m][:],
                        dt_sb[m][:, k:k + 16 * (NCH - 1) + 1:16],
                        op=mybir.AluOpType.add)
                    st_new.append(dst)
                st_prev = st_new

            # ---- phase B: banded combine  w_c = sum_p M_p b_{c-1-p} ----
            psw = [pp.tile([128, BCH], f32, tag="ps", name=f"psW{_m}") for _m in range(4)]
            for p in range(1, K):
                mbt = mbpool.tile([128, 4 * DZ], f16, tag="mbt")
                off = MBOFF + (p - 1) * 512
                nc.sync.dma_start(
                    mbt[:].rearrange("p (k n) -> p k n", k=4),
                    kon_d[off:off + 512, :].rearrange("(p k) n -> p k n", k=4))
                lo = K - 1 - p
                for m in range(4):
                    for kk in range(4):
                        nc.tensor.matmul(
                            psw[m][:],
                            mbt[:, 512 * kk + 128 * m:512 * kk + 128 * m + 128],
                            bmat[kk][:, lo:lo + BCH],
                            start=(p == 1 and kk == 0),
                            stop=(p == K - 1 and kk == 3))
            w_sb = []
            for m in range(4):
                wt = cpool.tile([128, BCH], f16, tag=f"w{m}", name=f"w{m}")
                nc.vector.tensor_tensor(
                    wt[:], psw[m][:], bmat[m][:, K - 1:K - 1 + BCH],
                    op=mybir.AluOpType.add)
                w_sb.append(wt)

            # ---- phase C: scan 256 chunks from w_c, fused output proj ----
            st_prev = w_sb
            for k in range(S):
                psl = [pp.tile([128, BCH], f32, tag="ps", name=f"psC{k}_{_m}") for _m in range(4)]
                for m in range(4):
                    for kk in range(4):
                        nc.tensor.matmul(
                            psl[m][:],
                            at_sb[kk][:, 128 * m:128 * (m + 1)],
                            st_prev[kk][:],
                            start=(kk == 0), stop=(kk == 3))
                st_new = []
                for m in range(4):
                    dst = stpool.tile([128, BCH], f16, tag=f"sc{m}", name=f"stC{k}_{m}")
                    nc.vector.tensor_tensor(
                        dst[:], psl[m][:],
                        dt_sb[m][:, H + k:H + k + 16 * (BCH - 1) + 1:16],
                        op=mybir.AluOpType.add)
                    st_new.append(dst)
                st_prev = st_new
                # output rows t = 16*c + k, int8 with per-row abs-max scale
                # (HW f32->int8 conversion rounds-to-nearest and saturates;
                # CoreSim truncates/wraps, so sim overreports quant error)
                for h in range(BCH // 128):
                    pso = pp.tile([128, DZ], f32, tag="ps")
                    for kk in range(4):
                        nc.tensor.matmul(
                            pso[:],
                            st_new[kk][:, 128 * h:128 * (h + 1)],
                            bt_sb[kk][:],
                            start=(kk == 0), stop=(kk == 3))
                    obf = opool.tile([128, DZ], f32, tag="ob")
                    nc.vector.tensor_tensor(
                        obf[:], pso[:], mn_sb[:], op=mybir.AluOpType.add)
                    amax = scpool.tile([128, 1], f32, tag="am")
                    nc.vector.tensor_reduce(
                        amax[:], obf[:], axis=mybir.AxisListType.X,
                        op=mybir.AluOpType.max, apply_absolute_value=True)
                    inv = scpool.tile([128, 1], f32, tag="iv")
                    nc.vector.reciprocal(inv[:], amax[:])
                    qt = opool.tile([128, OW], i8, tag="qt")
                    nc.vector.tensor_scalar(
                        qt[:, 0:DZ], obf[:], inv[:], 127.0,
                        op0=mybir.AluOpType.mult, op1=mybir.AluOpType.mult)
                    # pack the f32 scale into the last 4 int8 columns
                    nc.vector.tensor_copy(
                        qt[:, DZ:OW].bitcast(f32), amax[:])
                    r0 = 2048 * h + k
                    nc.sync.dma_start(out_d[r0:r0 + 2033:16, :], qt[:])
    nc.compile()
    return nc


def _build():
    """Compile the bass module + jit executable once; reuse across calls."""
    if "exe" in _CACHE:
        return _CACHE["exe"]

    install_neuronx_cc_hook()
    nc = bacc.Bacc("TRN2", target_bir_lowering=False, debug=False)
    _emit(nc)

    # in/out names in BIR allocation order (mirrors run_bass_via_pjrt):
    # partition_id is excluded here and appended as the LAST operand,
    # supplied on-device by the PartitionIdOp primitive.
    part_name = nc.partition_id_tensor.name if nc.partition_id_tensor else None
    in_names, out_names, out_avals = [], [], []
    for alloc in nc.m.functions[0].allocations:
        if not isinstance(alloc, mybir.MemoryLocationSet):
            continue
        name = alloc.memorylocations[0].name
        if alloc.kind == "ExternalInput":
            if name != part_name:
                in_names.append(name)
        elif alloc.kind == "ExternalOutput":
            out_names.append(name)
            out_avals.append(jax.core.ShapedArray(
                tuple(alloc.tensor_shape), mybir.dt.np(alloc.dtype)))
    assert in_names == ["u", "kon"], in_names
    assert out_names == ["out"], out_names
    all_names = tuple(in_names) + tuple(out_names)
    if part_name is not None:
        all_names = all_names + (part_name,)

    devs = jax.devices()[:NCORE]
    mesh = Mesh(np.asarray(devs), ("core",))
    sh_core = NamedSharding(mesh, P("core"))
    sh_rep = NamedSharding(mesh, P())

    def _body(u, kon, outz):
        operands = [u, kon, outz]
        if part_name is not None:
            operands.append(partition_id_tensor())
        outs = _bass_exec_p.bind(
            *operands,
            out_avals=tuple(out_avals),
            in_names=all_names,
            out_names=tuple(out_names),
            lowering_input_output_aliases=(),
            sim_require_finite=True,
            sim_require_nnan=True,
            nc=nc)
        return tuple(outs)

    sharded = jax.jit(
        shard_map(_body, mesh=mesh,
                  in_specs=(P("core"), P(), P("core")),
                  out_specs=(P("core"),), check_rep=False),
        donate_argnums=(2,), keep_unused=True)
    zmaker = jax.jit(lambda: jnp.zeros((NCORE * TLOC, OW), jnp.int8),
                     out_shardings=sh_core)

    exe = {"sharded": sharded, "zmaker": zmaker, "devs": devs,
           "sh_core": sh_core, "sh_rep": sh_rep}
    _CACHE["exe"] = exe
    return exe


def _make_kon(mean, A, B, C, ucol):
    """Packed fp16 constants; u int8 scales are folded into C.T rows."""
    AS = np.linalg.matrix_power(A, S)
    kon = np.empty((KROWS, DZ), np.float16)
    kon[0:512] = A.T
    kon[512:1024] = B.T
    kon[1024:1280] = C.T * (ucol / np.float32(127.0))[:, None]
    kon[1280:1408] = np.broadcast_to(mean, (128, DZ))
    Mp = AS.copy()
    for p in range(1, K):
        off = MBOFF + (p - 1) * 512
        kon[off:off + 512] = (
            Mp.T.reshape(4, 128, DZ).transpose(1, 0, 2).reshape(512, DZ))
        Mp = Mp @ AS
    return kon


_POOL = ThreadPoolExecutor(max_workers=8)


def _quant_u(inputs_np, ucol):
    """int8-quantize inputs per feature column, transpose, add halos."""
    uinv = np.float32(127.0) / ucol
    inT = inputs_np.T
    uqT = np.empty((DU, T), np.int8)

    def qblock(r0, r1):
        uqT[r0:r1] = np.rint(inT[r0:r1] * uinv[r0:r1, None])

    blocks = [(r, r + 32) for r in range(0, DU, 32)]
    list(_POOL.map(lambda b: qblock(*b), blocks))
    ug = np.zeros((NCORE * DU, ULEN), np.int8)
    for i in range(NCORE):
        lo = i * TLOC - H
        s = max(0, -lo)
        ug[i * DU:(i + 1) * DU, s:] = uqT[:, lo + s:i * TLOC + TLOC]
    return ug


def kernel(data, inputs, mean, A, B, C, recognition_matrix, steps=None, **kw):
    data = np.asarray(data, np.float32)
    inputs_np = np.asarray(inputs, np.float32)
    mean = np.asarray(mean, np.float32)
    A = np.asarray(A, np.float32)
    B = np.asarray(B, np.float32)
    C = np.asarray(C, np.float32)
    R = np.asarray(recognition_matrix, np.float32)

    exe = _build()
    outz = exe["zmaker"]()                      # async, on-device zeros

    ucol = np.abs(inputs_np).max(axis=0)
    # constants are cached on device across calls keyed by content; any
    # change in A/B/C/mean/input scales recomputes and re-uploads
    kh = hashlib.blake2b(
        A.tobytes() + B.tobytes() + C.tobytes() + mean.tobytes()
        + ucol.tobytes(), digest_size=16).hexdigest()
    if _CACHE.get("kon_key") != kh:
        kon = _make_kon(mean, A, B, C, ucol)
        kon0 = jax.device_put(kon, exe["devs"][0])
        _CACHE["kon_rep"] = jax.device_put(kon0, exe["sh_rep"])
        _CACHE["kon_key"] = kh
    kon_rep = _CACHE["kon_rep"]

    ug = _quant_u(inputs_np, ucol)              # overlaps kon upload
    u_dev = jax.device_put(ug, exe["sh_core"])
    try:
        (out_dev,) = exe["sharded"](u_dev, kon_rep, outz)
    except Exception:
        # one retry: a previously crashed process can leave the exec unit
        # wedged; the failed attempt resets it
        outz = exe["zmaker"]()
        (out_dev,) = exe["sharded"](u_dev, kon_rep, outz)

    # host correction for rows 0..H-1 while the result streams back:
    #   out row n-1 += (A^n z0) @ B.T for n = 1..H
    z0 = R @ (data[0] - mean[0])
    zc = z0
    corr = np.empty((H, DZ), np.float32)
    for n in range(1, H + 1):
        zc = A @ zc
        corr[n - 1] = B @ zc

    buf = np.asarray(out_dev)                   # blocks on D2H
    scale = buf[:, DZ:OW].copy().view(np.float32) * np.float32(1.0 / 127.0)
    out = np.empty((T, DZ), np.float32)

    def dq(r0, r1):
        np.multiply(buf[r0:r1, 0:DZ], scale[r0:r1], out=out[r0:r1])

    list(_POOL.map(lambda b: dq(*b), [(r, r + TLOC) for r in range(0, T, TLOC)]))
    out[:H] += corr
    return out


# revision 17
# speedup vs baseline: 1.0697x; 1.0697x over previous
"""Trainium2 Bass kernel for the KalmanFilter linear recurrence.

  x = data - mean;  z0 = R @ x[0];  drive = inputs @ C.T
  z_{t+1} = A z_t + drive[t]   (T = 32768 steps, dim 512)
  result  = Z[1:] @ B.T + mean

Strategy (8 NeuronCores, sequence-parallel, no collectives):
  - ||A^k|| decays like 0.9^k (spectral radius 0.9), so the recurrence
    forgets its state after H=128 steps to ~1e-5 relative.
  - Each core owns 4096 contiguous steps, split into 256 chunks of S=16
    steps + K=8 extra "halo" chunks covering the preceding H=128 steps.
  - Phase A: batched zero-init scan over all 264 chunks (state tiles
    [512, 264], 15 matmul steps) -> per-chunk accumulated drives b_c.
  - Phase B: chunk-start states w_c = sum_{p=0}^{K-1} (A^16)^p b_{c-1-p}
    (banded combine; truncated at ||A^128|| ~ 4e-4 of a unit).
  - Phase C: re-scan the 256 real chunks from inits w_c; each step also
    applies the output projection B.T (+mean) and streams rows to DRAM.
  - z0 only affects output rows 0..H-1 (through A^n z0); that correction
    is added on the host, so the device never sees `data`/`R`.

  Wall time is dominated by the host<->device tunnel (~55MB/s), so the
  wire format is aggressively compressed: inputs ship as int8 with
  per-feature scales folded into C.T on the host; outputs ship as int8
  rows with a per-row abs-max scale (f32) packed into 4 extra int8
  columns of the same tensor. Matrix constants ship fp16, packed into
  one tensor uploaded to core 0, broadcast device-to-device, and cached
  on device across calls keyed by a content hash. Donated output zero
  buffers are created on device. Matmuls run fp16 with f32 PSUM
  accumulation. The jit executable is built once and cached.
"""
import hashlib
import numpy as np
import jax
import jax.numpy as jnp
from jax.experimental.shard_map import shard_map
from jax.sharding import Mesh, NamedSharding, PartitionSpec as P

import concourse.bacc as bacc
import concourse.mybir as mybir
from concourse import tile
from concourse.bass2jax import (
    _bass_exec_p, install_neuronx_cc_hook, partition_id_tensor)

T = 32768
DZ = 512
DU = 256
NCORE = 8
TLOC = T // NCORE          # 4096
S = 16                     # steps per chunk
BCH = TLOC // S            # 256 chunks per core
H = 128                    # halo steps (forgetting horizon)
K = H // S                 # 8 banded taps (incl. identity)
NCH = BCH + K              # 264 chunks in phase A
ULEN = TLOC + H            # 4224 drive rows per core (multiple of 128)
OW = DZ + 4                # 516: int8 row + 4 bytes of f32 row scale
# packed constants: at(512) bt(512) ct(256) mn(128) mb(7*512)
KROWS = 512 + 512 + 256 + 128 + (K - 1) * 512   # 4992
MBOFF = 1408

f16 = mybir.dt.float16
f32 = mybir.dt.float32
i8 = mybir.dt.int8

_CACHE = {}


def _emit(nc):
    u_d = nc.dram_tensor("u", (2 * 128, ULEN), i8, kind="ExternalInput")
    kon_d = nc.dram_tensor("kon", (KROWS, DZ), f16, kind="ExternalInput")
    out_d = nc.dram_tensor("out", (TLOC, OW), i8, kind="ExternalOutput")

    with tile.TileContext(nc) as tc:
        with tc.tile_pool(name="const", bufs=1) as cpool, \
             tc.tile_pool(name="dt", bufs=1) as dpool, \
             tc.tile_pool(name="ut", bufs=1) as upool, \
             tc.tile_pool(name="mb", bufs=3) as mbpool, \
             tc.tile_pool(name="st", bufs=2) as stpool, \
             tc.tile_pool(name="ob", bufs=4) as opool, \
             tc.tile_pool(name="sc", bufs=8) as scpool, \
             tc.tile_pool(name="ps", bufs=8, space="PSUM") as pp:

            # ---- constant loads (packed rows of kon) ----
            at_sb = [cpool.tile([128, DZ], f16, tag=f"at{k}", name=f"at{k}") for k in range(4)]
            bt_sb = [cpool.tile([128, DZ], f16, tag=f"bt{k}", name=f"bt{k}") for k in range(4)]
            ct_sb = [cpool.tile([128, DZ], f16, tag=f"ct{k}", name=f"ct{k}") for k in range(2)]
            mn_sb = cpool.tile([128, DZ], f16, tag="mn")
            for k in range(4):
                nc.sync.dma_start(at_sb[k][:], kon_d[128 * k:128 * (k + 1), :])
            for k in range(4):
                nc.sync.dma_start(bt_sb[k][:], kon_d[512 + 128 * k:512 + 128 * (k + 1), :])
            for k in range(2):
                nc.sync.dma_start(ct_sb[k][:], kon_d[1024 + 128 * k:1024 + 128 * (k + 1), :])
            nc.sync.dma_start(mn_sb[:], kon_d[1280:1408, :])

            # u.T tiles (int8 on the wire, widened to fp16 for the PE)
            uq_sb = [upool.tile([128, ULEN], i8, tag=f"uq{k}", name=f"uq{k}") for k in range(2)]
            ut_sb = [upool.tile([128, ULEN], f16, tag=f"ut{k}", name=f"ut{k}") for k in range(2)]
            for k in range(2):
                nc.sync.dma_start(uq_sb[k][:], u_d[128 * k:128 * (k + 1), :])
            for k in range(2):
                nc.vector.tensor_copy(ut_sb[k][:], uq_sb[k][:])

            # drive rows (transposed): dt[m] holds drive.T[128m:128(m+1), :]
            dt_sb = [dpool.tile([128, ULEN], f16, tag=f"dt{m}", name=f"dt{m}") for m in range(4)]
            for nb in range((ULEN + 511) // 512):
                nb0 = nb * 512
                w = min(512, ULEN - nb0)
                for m in range(4):
                    psd = pp.tile([128, 512], f32, tag="ps")
                    for kk in range(2):
                        nc.tensor.matmul(
                            psd[:, :w],
                            ct_sb[kk][:, 128 * m:128 * (m + 1)],
                            ut_sb[kk][:, nb0:nb0 + w],
                            start=(kk == 0), stop=(kk == 1))
                    nc.any.tensor_copy(dt_sb[m][:, nb0:nb0 + w], psd[:, :w])

            # ---- phase A: zero-init scan over NCH chunks ----
            bmat = [cpool.tile([128, NCH], f16, tag=f"bm{m}", name=f"bm{m}") for m in range(4)]
            st_prev = []
            for m in range(4):
                t0 = stpool.tile([128, NCH], f16, tag=f"st{m}", name=f"st0_{m}")
                nc.vector.tensor_copy(t0[:], dt_sb[m][:, 0:16 * (NCH - 1) + 1:16])
                st_prev.append(t0)
            for k in range(1, S):
                psl = [pp.tile([128, NCH], f32, tag="ps", name=f"psA{k}_{_m}") for _m in range(4)]
                for m in range(4):
                    for kk in range(4):
                        nc.tensor.matmul(
                            psl[m][:],
                            at_sb[kk][:, 128 * m:128 * (m + 1)],
                            st_prev[kk][:],
                            start=(kk == 0), stop=(kk == 3))
                st_new = []
                for m in range(4):
                    dst = (bmat[m] if k == S - 1 else
                           stpool.tile([128, NCH], f16, tag=f"st{m}", name=f"stA{k}_{m}"))
                    nc.vector.tensor_tensor(
                        dst[:], psl[m][:],
                        dt_sb[m][:, k:k + 16 * (NCH - 1) + 1:16],
                        op=mybir.AluOpType.add)
                    st_new.append(dst)
                st_prev = st_new

            # ---- phase B: banded combine  w_c = sum_p M_p b_{c-1-p} ----
            psw = [pp.tile([128, BCH], f32, tag="ps", name=f"psW{_m}") for _m in range(4)]
            for p in range(1, K):
                mbt = mbpool.tile([128, 4 * DZ], f16, tag="mbt")
                off = MBOFF + (p - 1) * 512
                nc.sync.dma_start(
                    mbt[:].rearrange("p (k n) -> p k n", k=4),
                    kon_d[off:off + 512, :].rearrange("(p k) n -> p k n", k=4))
                lo = K - 1 - p
                for m in range(4):
                    for kk in range(4):
                        nc.tensor.matmul(
                            psw[m][:],
                            mbt[:, 512 * kk + 128 * m:512 * kk + 128 * m + 128],
                            bmat[kk][:, lo:lo + BCH],
                            start=(p == 1 and kk == 0),
                            stop=(p == K - 1 and kk == 3))
            w_sb = []
            for m in range(4):
                wt = cpool.tile([128, BCH], f16, tag=f"w{m}", name=f"w{m}")
                nc.vector.tensor_tensor(
                    wt[:], psw[m][:], bmat[m][:, K - 1:K - 1 + BCH],
                    op=mybir.AluOpType.add)
                w_sb.append(wt)

            # ---- phase C: scan 256 chunks from w_c, fused output proj ----
            st_prev = w_sb
            for k in range(S):
                psl = [pp.tile([128, BCH], f32, tag="ps", name=f"psC{k}_{_m}") for _m in range(4)]
                for m in range(4):
                    for kk in range(4):
                        nc.tensor.matmul(
                            psl[m][:],
                            at_sb[kk][:, 128 * m:128 * (m + 1)],
                            st_prev[kk][:],
                            start=(kk == 0), stop=(kk == 3))
                st_new = []
                for m in range(4):
                    dst = stpool.tile([128, BCH], f16, tag=f"sc{m}", name=f"stC{k}_{m}")
                    nc.vector.tensor_tensor(
                        dst[:], psl[m][:],
                        dt_sb[m][:, H + k:H + k + 16 * (BCH - 1) + 1:16],
                        op=mybir.AluOpType.add)
                    st_new.append(dst)
                st_prev = st_new
                # output rows t = 16*c + k, int8 with per-row abs-max scale
                # (HW f32->int8 conversion rounds-to-nearest and saturates;
                # CoreSim truncates/wraps, so sim overreports quant error)
                for h in range(BCH // 128):
                    pso = pp.tile([128, DZ], f32, tag="ps")
                    for kk in range(4):
                        nc.tensor.matmul(
                            pso[:],
                            st_new[kk][:, 128 * h:128 * (h + 1)],
                            bt_sb[kk][:],
                            start=(kk == 0), stop=(kk == 3))
                    obf = opool.tile([128, DZ], f32, tag="ob")
                    nc.vector.tensor_tensor(
                        obf[:], pso[:], mn_sb[:], op=mybir.AluOpType.add)
                    amax = scpool.tile([128, 1], f32, tag="am")
                    nc.vector.tensor_reduce(
                        amax[:], obf[:], axis=mybir.AxisListType.X,
                        op=mybir.AluOpType.max, apply_absolute_value=True)
                    inv = scpool.tile([128, 1], f32, tag="iv")
                    nc.vector.reciprocal(inv[:], amax[:])
                    qt = opool.tile([128, OW], i8, tag="qt")
                    nc.vector.tensor_scalar(
                        qt[:, 0:DZ], obf[:], inv[:], 127.0,
                        op0=mybir.AluOpType.mult, op1=mybir.AluOpType.mult)
                    # pack the f32 scale into the last 4 int8 columns
                    nc.vector.tensor_copy(
                        qt[:, DZ:OW].bitcast(f32), amax[:])
                    r0 = 2048 * h + k
                    nc.sync.dma_start(out_d[r0:r0 + 2033:16, :], qt[:])
    nc.compile()
    return nc


def _build():
    """Compile the bass module + jit executable once; reuse across calls."""
    if "exe" in _CACHE:
        return _CACHE["exe"]

    install_neuronx_cc_hook()
    nc = bacc.Bacc("TRN2", target_bir_lowering=False, debug=False)
    _emit(nc)

    # in/out names in BIR allocation order (mirrors run_bass_via_pjrt):
    # partition_id is excluded here and appended as the LAST operand,
    # supplied on-device by the PartitionIdOp primitive.
    part_name = nc.partition_id_tensor.name if nc.partition_id_tensor else None
    in_names, out_names, out_avals = [], [], []
    for alloc in nc.m.functions[0].allocations:
        if not isinstance(alloc, mybir.MemoryLocationSet):
            continue
        name = alloc.memorylocations[0].name
        if alloc.kind == "ExternalInput":
            if name != part_name:
                in_names.append(name)
        elif alloc.kind == "ExternalOutput":
            out_names.append(name)
            out_avals.append(jax.core.ShapedArray(
                tuple(alloc.tensor_shape), mybir.dt.np(alloc.dtype)))
    assert in_names == ["u", "kon"], in_names
    assert out_names == ["out"], out_names
    all_names = tuple(in_names) + tuple(out_names)
    if part_name is not None:
        all_names = all_names + (part_name,)

    devs = jax.devices()[:NCORE]
    mesh = Mesh(np.asarray(devs), ("core",))
    sh_core = NamedSharding(mesh, P("core"))
    sh_rep = NamedSharding(mesh, P())

    def _body(u, kon, outz):
        operands = [u, kon, outz]
        if part_name is not None:
            operands.append(partition_id_tensor())
        outs = _bass_exec_p.bind(
            *operands,
            out_avals=tuple(out_avals),
            in_names=all_names,
            out_names=tuple(out_names),
            lowering_input_output_aliases=(),
            sim_require_finite=True,
            sim_require_nnan=True,
            nc=nc)
        return tuple(outs)

    sharded = jax.jit(
        shard_map(_body, mesh=mesh,
                  in_specs=(P("core"), P(), P("core")),
                  out_specs=(P("core"),), check_rep=False),
        donate_argnums=(2,), keep_unused=True)
    zmaker = jax.jit(lambda: jnp.zeros((NCORE * TLOC, OW), jnp.int8),
                     out_shardings=sh_core)

    exe = {"sharded": sharded, "zmaker": zmaker, "devs": devs,
           "sh_core": sh_core, "sh_rep": sh_rep}
    _CACHE["exe"] = exe
    return exe


def _make_kon(mean, A, B, C, ucol):
    """Packed fp16 constants; u int8 scales are folded into C.T rows."""
    AS = np.linalg.matrix_power(A, S)
    kon = np.empty((KROWS, DZ), np.float16)
    kon[0:512] = A.T
    kon[512:1024] = B.T
    kon[1024:1280] = C.T * (ucol / np.float32(127.0))[:, None]
    kon[1280:1408] = np.broadcast_to(mean, (128, DZ))
    Mp = AS.copy()
    for p in range(1, K):
        off = MBOFF + (p - 1) * 512
        kon[off:off + 512] = (
            Mp.T.reshape(4, 128, DZ).transpose(1, 0, 2).reshape(512, DZ))
        Mp = Mp @ AS
    return kon


def _quant_u(inputs_np, ucol):
    """int8-quantize inputs per feature column, transpose, add halos."""
    uinv = np.float32(127.0) / ucol
    uqT = np.rint(inputs_np.T * uinv[:, None]).astype(np.int8)   # (DU, T)
    ug = np.zeros((NCORE * DU, ULEN), np.int8)
    for i in range(NCORE):
        lo = i * TLOC - H
        s = max(0, -lo)
        ug[i * DU:(i + 1) * DU, s:] = uqT[:, lo + s:i * TLOC + TLOC]
    return ug


def kernel(data, inputs, mean, A, B, C, recognition_matrix, steps=None, **kw):
    data = np.asarray(data, np.float32)
    inputs_np = np.asarray(inputs, np.float32)
    mean = np.asarray(mean, np.float32)
    A = np.asarray(A, np.float32)
    B = np.asarray(B, np.float32)
    C = np.asarray(C, np.float32)
    R = np.asarray(recognition_matrix, np.float32)

    exe = _build()
    outz = exe["zmaker"]()                      # async, on-device zeros

    ucol = np.abs(inputs_np).max(axis=0)
    # constants are cached on device across calls keyed by content; any
    # change in A/B/C/mean/input scales recomputes and re-uploads
    kh = hashlib.blake2b(
        A.tobytes() + B.tobytes() + C.tobytes() + mean.tobytes()
        + ucol.tobytes(), digest_size=16).hexdigest()
    if _CACHE.get("kon_key") != kh:
        kon = _make_kon(mean, A, B, C, ucol)
        kon0 = jax.device_put(kon, exe["devs"][0])
        _CACHE["kon_rep"] = jax.device_put(kon0, exe["sh_rep"])
        _CACHE["kon_key"] = kh
    kon_rep = _CACHE["kon_rep"]

    ug = _quant_u(inputs_np, ucol)              # overlaps kon upload
    u_dev = jax.device_put(ug, exe["sh_core"])
    try:
        (out_dev,) = exe["sharded"](u_dev, kon_rep, outz)
    except Exception:
        # one retry: a previously crashed process can leave the exec unit
        # wedged; the failed attempt resets it
        outz = exe["zmaker"]()
        (out_dev,) = exe["sharded"](u_dev, kon_rep, outz)

    # host correction while the result streams back: out row n-1 +=
    # (A^n z0) @ B.T.  ||A^n z0|| ~ 0.9^n, so 64 rows reach ~1e-3 of a
    # unit (well under the int8 quantization noise).
    HC = 64
    z0 = R @ (data[0] - mean[0])
    zc = z0
    corr = np.empty((HC, DZ), np.float32)
    for n in range(1, HC + 1):
        zc = A @ zc
        corr[n - 1] = B @ zc

    buf = np.asarray(out_dev)                   # blocks on D2H
    scale = buf[:, DZ:OW].copy().view(np.float32) * np.float32(1.0 / 127.0)
    out = np.empty((T, DZ), np.float32)
    np.multiply(buf[:, 0:DZ], scale, out=out)
    out[:HC] += corr
    return out


# revision 19
# speedup vs baseline: 1.0796x; 1.0092x over previous
"""Trainium2 Bass kernel for the KalmanFilter linear recurrence.

  x = data - mean;  z0 = R @ x[0];  drive = inputs @ C.T
  z_{t+1} = A z_t + drive[t]   (T = 32768 steps, dim 512)
  result  = Z[1:] @ B.T + mean

Strategy (8 NeuronCores, sequence-parallel, no collectives):
  - ||A^k|| decays like 0.9^k (spectral radius 0.9), so the recurrence
    forgets its state after H=128 steps to ~1e-5 relative.
  - Each core owns 4096 contiguous steps, split into 256 chunks of S=16
    steps + K=8 extra "halo" chunks covering the preceding H=128 steps.
  - Phase A: batched zero-init scan over all 264 chunks (state tiles
    [512, 264], 15 matmul steps) -> per-chunk accumulated drives b_c.
  - Phase B: chunk-start states w_c = sum_{p=0}^{K-1} (A^16)^p b_{c-1-p}
    (banded combine; truncated at ||A^128|| ~ 4e-4 of a unit).
  - Phase C: re-scan the 256 real chunks from inits w_c; each step also
    applies the output projection B.T (+mean) and streams rows to DRAM.
  - z0 only affects output rows 0..H-1 (through A^n z0); that correction
    is added on the host, so the device never sees `data`/`R`.

  Wall time is dominated by the host<->device tunnel (~55MB/s), so the
  wire format is aggressively compressed: inputs ship as int8 with
  per-feature scales folded into C.T on the host; outputs ship as int8
  rows with a per-row abs-max scale (f32) packed into 4 extra int8
  columns of the same tensor. Matrix constants ship fp16, packed into
  one tensor uploaded to core 0, broadcast device-to-device, and cached
  on device across calls keyed by a content hash. Donated output zero
  buffers are created on device. Matmuls run fp16 with f32 PSUM
  accumulation. The jit executable is built once and cached.
"""
import hashlib
import numpy as np
import jax
import jax.numpy as jnp
from jax.experimental.shard_map import shard_map
from jax.sharding import Mesh, NamedSharding, PartitionSpec as P

import concourse.bacc as bacc
import concourse.mybir as mybir
from concourse import tile
from concourse.bass2jax import (
    _bass_exec_p, install_neuronx_cc_hook, partition_id_tensor)

T = 32768
DZ = 512
DU = 256
NCORE = 8
TLOC = T // NCORE          # 4096
S = 16                     # steps per chunk
BCH = TLOC // S            # 256 chunks per core
H = 128                    # halo steps (forgetting horizon)
K = H // S                 # 8 banded taps (incl. identity)
NCH = BCH + K              # 264 chunks in phase A
ULEN = TLOC + H            # 4224 drive rows per core (multiple of 128)
OW = DZ + 4                # 516: int8 row + 4 bytes of f32 row scale
# packed constants: at(512) bt(512) ct(256) mn(128) mb(7*512)
KROWS = 512 + 512 + 256 + 128 + (K - 1) * 512   # 4992
MBOFF = 1408

f16 = mybir.dt.float16
f32 = mybir.dt.float32
i8 = mybir.dt.int8

_CACHE = {}


def _emit(nc):
    u_d = nc.dram_tensor("u", (2 * 128, ULEN), i8, kind="ExternalInput")
    kon_d = nc.dram_tensor("kon", (KROWS, DZ), f16, kind="ExternalInput")
    out_d = nc.dram_tensor("out", (TLOC, OW), i8, kind="ExternalOutput")

    with tile.TileContext(nc) as tc:
        with tc.tile_pool(name="const", bufs=1) as cpool, \
             tc.tile_pool(name="dt", bufs=1) as dpool, \
             tc.tile_pool(name="ut", bufs=1) as upool, \
             tc.tile_pool(name="mb", bufs=3) as mbpool, \
             tc.tile_pool(name="st", bufs=2) as stpool, \
             tc.tile_pool(name="ob", bufs=4) as opool, \
             tc.tile_pool(name="sc", bufs=8) as scpool, \
             tc.tile_pool(name="ps", bufs=8, space="PSUM") as pp:

            # ---- constant loads (packed rows of kon) ----
            at_sb = [cpool.tile([128, DZ], f16, tag=f"at{k}", name=f"at{k}") for k in range(4)]
            bt_sb = [cpool.tile([128, DZ], f16, tag=f"bt{k}", name=f"bt{k}") for k in range(4)]
            ct_sb = [cpool.tile([128, DZ], f16, tag=f"ct{k}", name=f"ct{k}") for k in range(2)]
            mn_sb = cpool.tile([128, DZ], f16, tag="mn")
            for k in range(4):
                nc.sync.dma_start(at_sb[k][:], kon_d[128 * k:128 * (k + 1), :])
            for k in range(4):
                nc.sync.dma_start(bt_sb[k][:], kon_d[512 + 128 * k:512 + 128 * (k + 1), :])
            for k in range(2):
                nc.sync.dma_start(ct_sb[k][:], kon_d[1024 + 128 * k:1024 + 128 * (k + 1), :])
            nc.sync.dma_start(mn_sb[:], kon_d[1280:1408, :])

            # u.T tiles (int8 on the wire, widened to fp16 for the PE)
            uq_sb = [upool.tile([128, ULEN], i8, tag=f"uq{k}", name=f"uq{k}") for k in range(2)]
            ut_sb = [upool.tile([128, ULEN], f16, tag=f"ut{k}", name=f"ut{k}") for k in range(2)]
            for k in range(2):
                nc.sync.dma_start(uq_sb[k][:], u_d[128 * k:128 * (k + 1), :])
            for k in range(2):
                nc.vector.tensor_copy(ut_sb[k][:], uq_sb[k][:])

            # drive rows (transposed): dt[m] holds drive.T[128m:128(m+1), :]
            dt_sb = [dpool.tile([128, ULEN], f16, tag=f"dt{m}", name=f"dt{m}") for m in range(4)]
            for nb in range((ULEN + 511) // 512):
                nb0 = nb * 512
                w = min(512, ULEN - nb0)
                for m in range(4):
                    psd = pp.tile([128, 512], f32, tag="ps")
                    for kk in range(2):
                        nc.tensor.matmul(
                            psd[:, :w],
                            ct_sb[kk][:, 128 * m:128 * (m + 1)],
                            ut_sb[kk][:, nb0:nb0 + w],
                            start=(kk == 0), stop=(kk == 1))
                    nc.any.tensor_copy(dt_sb[m][:, nb0:nb0 + w], psd[:, :w])

            # ---- phase A: zero-init scan over NCH chunks ----
            bmat = [cpool.tile([128, NCH], f16, tag=f"bm{m}", name=f"bm{m}") for m in range(4)]
            st_prev = []
            for m in range(4):
                t0 = stpool.tile([128, NCH], f16, tag=f"st{m}", name=f"st0_{m}")
                nc.vector.tensor_copy(t0[:], dt_sb[m][:, 0:16 * (NCH - 1) + 1:16])
                st_prev.append(t0)
            for k in range(1, S):
                psl = [pp.tile([128, NCH], f32, tag="ps", name=f"psA{k}_{_m}") for _m in range(4)]
                for m in range(4):
                    for kk in range(4):
                        nc.tensor.matmul(
                            psl[m][:],
                            at_sb[kk][:, 128 * m:128 * (m + 1)],
                            st_prev[kk][:],
                            start=(kk == 0), stop=(kk == 3))
                st_new = []
                for m in range(4):
                    dst = (bmat[m] if k == S - 1 else
                           stpool.tile([128, NCH], f16, tag=f"st{m}", name=f"stA{k}_{m}"))
                    nc.vector.tensor_tensor(
                        dst[:], psl[m][:],
                        dt_sb[m][:, k:k + 16 * (NCH - 1) + 1:16],
                        op=mybir.AluOpType.add)
                    st_new.append(dst)
                st_prev = st_new

            # ---- phase B: banded combine  w_c = sum_p M_p b_{c-1-p} ----
            psw = [pp.tile([128, BCH], f32, tag="ps", name=f"psW{_m}") for _m in range(4)]
            for p in range(1, K):
                mbt = mbpool.tile([128, 4 * DZ], f16, tag="mbt")
                off = MBOFF + (p - 1) * 512
                nc.sync.dma_start(
                    mbt[:].rearrange("p (k n) -> p k n", k=4),
                    kon_d[off:off + 512, :].rearrange("(p k) n -> p k n", k=4))
                lo = K - 1 - p
                for m in range(4):
                    for kk in range(4):
                        nc.tensor.matmul(
                            psw[m][:],
                            mbt[:, 512 * kk + 128 * m:512 * kk + 128 * m + 128],
                            bmat[kk][:, lo:lo + BCH],
                            start=(p == 1 and kk == 0),
                            stop=(p == K - 1 and kk == 3))
            w_sb = []
            for m in range(4):
                wt = cpool.tile([128, BCH], f16, tag=f"w{m}", name=f"w{m}")
                nc.vector.tensor_tensor(
                    wt[:], psw[m][:], bmat[m][:, K - 1:K - 1 + BCH],
                    op=mybir.AluOpType.add)
                w_sb.append(wt)

            # ---- phase C: scan 256 chunks from w_c, fused output proj ----
            st_prev = w_sb
            for k in range(S):
                psl = [pp.tile([128, BCH], f32, tag="ps", name=f"psC{k}_{_m}") for _m in range(4)]
                for m in range(4):
                    for kk in range(4):
                        nc.tensor.matmul(
                            psl[m][:],
                            at_sb[kk][:, 128 * m:128 * (m + 1)],
                            st_prev[kk][:],
                            start=(kk == 0), stop=(kk == 3))
                st_new = []
                for m in range(4):
                    dst = stpool.tile([128, BCH], f16, tag=f"sc{m}", name=f"stC{k}_{m}")
                    nc.vector.tensor_tensor(
                        dst[:], psl[m][:],
                        dt_sb[m][:, H + k:H + k + 16 * (BCH - 1) + 1:16],
                        op=mybir.AluOpType.add)
                    st_new.append(dst)
                st_prev = st_new
                # output rows t = 16*c + k, int8 with per-row abs-max scale
                # (HW f32->int8 conversion rounds-to-nearest and saturates;
                # CoreSim truncates/wraps, so sim overreports quant error)
                for h in range(BCH // 128):
                    pso = pp.tile([128, DZ], f32, tag="ps")
                    for kk in range(4):
                        nc.tensor.matmul(
                            pso[:],
                            st_new[kk][:, 128 * h:128 * (h + 1)],
                            bt_sb[kk][:],
                            start=(kk == 0), stop=(kk == 3))
                    obf = opool.tile([128, DZ], f32, tag="ob")
                    nc.vector.tensor_tensor(
                        obf[:], pso[:], mn_sb[:], op=mybir.AluOpType.add)
                    amax = scpool.tile([128, 1], f32, tag="am")
                    nc.vector.tensor_reduce(
                        amax[:], obf[:], axis=mybir.AxisListType.X,
                        op=mybir.AluOpType.max, apply_absolute_value=True)
                    inv = scpool.tile([128, 1], f32, tag="iv")
                    nc.vector.reciprocal(inv[:], amax[:])
                    qt = opool.tile([128, OW], i8, tag="qt")
                    nc.vector.tensor_scalar(
                        qt[:, 0:DZ], obf[:], inv[:], 127.0,
                        op0=mybir.AluOpType.mult, op1=mybir.AluOpType.mult)
                    # pack the f32 scale into the last 4 int8 columns
                    nc.vector.tensor_copy(
                        qt[:, DZ:OW].bitcast(f32), amax[:])
                    r0 = 2048 * h + k
                    nc.sync.dma_start(out_d[r0:r0 + 2033:16, :], qt[:])
    nc.compile()
    return nc


def _build():
    """Compile the bass module + jit executable once; reuse across calls."""
    if "exe" in _CACHE:
        return _CACHE["exe"]

    install_neuronx_cc_hook()
    nc = bacc.Bacc("TRN2", target_bir_lowering=False, debug=False)
    _emit(nc)

    # in/out names in BIR allocation order (mirrors run_bass_via_pjrt):
    # partition_id is excluded here and appended as the LAST operand,
    # supplied on-device by the PartitionIdOp primitive.
    part_name = nc.partition_id_tensor.name if nc.partition_id_tensor else None
    in_names, out_names, out_avals = [], [], []
    for alloc in nc.m.functions[0].allocations:
        if not isinstance(alloc, mybir.MemoryLocationSet):
            continue
        name = alloc.memorylocations[0].name
        if alloc.kind == "ExternalInput":
            if name != part_name:
                in_names.append(name)
        elif alloc.kind == "ExternalOutput":
            out_names.append(name)
            out_avals.append(jax.core.ShapedArray(
                tuple(alloc.tensor_shape), mybir.dt.np(alloc.dtype)))
    assert in_names == ["u", "kon"], in_names
    assert out_names == ["out"], out_names
    all_names = tuple(in_names) + tuple(out_names)
    if part_name is not None:
        all_names = all_names + (part_name,)

    devs = jax.devices()[:NCORE]
    mesh = Mesh(np.asarray(devs), ("core",))
    sh_core = NamedSharding(mesh, P("core"))
    sh_rep = NamedSharding(mesh, P())

    def _body(u, kon, outz):
        operands = [u, kon, outz]
        if part_name is not None:
            operands.append(partition_id_tensor())
        outs = _bass_exec_p.bind(
            *operands,
            out_avals=tuple(out_avals),
            in_names=all_names,
            out_names=tuple(out_names),
            lowering_input_output_aliases=(),
            sim_require_finite=True,
            sim_require_nnan=True,
            nc=nc)
        return tuple(outs)

    sharded = jax.jit(
        shard_map(_body, mesh=mesh,
                  in_specs=(P("core"), P(), P("core")),
                  out_specs=(P("core"),), check_rep=False),
        donate_argnums=(2,), keep_unused=True)
    zmaker = jax.jit(lambda: jnp.zeros((NCORE * TLOC, OW), jnp.int8),
                     out_shardings=sh_core)

    exe = {"sharded": sharded, "zmaker": zmaker, "devs": devs,
           "sh_core": sh_core, "sh_rep": sh_rep}
    _CACHE["exe"] = exe
    return exe


def _make_kon(mean, A, B, C, ucol):
    """Packed fp16 constants; u int8 scales are folded into C.T rows."""
    AS = np.linalg.matrix_power(A, S)
    kon = np.empty((KROWS, DZ), np.float16)
    kon[0:512] = A.T
    kon[512:1024] = B.T
    kon[1024:1280] = C.T * (ucol / np.float32(127.0))[:, None]
    kon[1280:1408] = np.broadcast_to(mean, (128, DZ))
    Mp = AS.copy()
    for p in range(1, K):
        off = MBOFF + (p - 1) * 512
        kon[off:off + 512] = (
            Mp.T.reshape(4, 128, DZ).transpose(1, 0, 2).reshape(512, DZ))
        Mp = Mp @ AS
    return kon


def _stream_u(inputs_np, ucol, exe):
    """Per-core int8 quantization streamed into per-device uploads, so
    the CPU quant of core i+1 overlaps the wire transfer of core i."""
    uinv = (np.float32(127.0) / ucol)[:, None]
    inT = inputs_np.T
    if "ubufs" not in _CACHE:
        # core 0's H-column halo stays zero across calls
        _CACHE["ubufs"] = [np.zeros((DU, ULEN), np.int8) for _ in range(NCORE)]
        _CACHE["utmp"] = np.empty((DU, ULEN), np.float32)
    tmp = _CACHE["utmp"]
    shards = []
    for i in range(NCORE):
        lo = i * TLOC - H
        s = max(0, -lo)
        w = ULEN - s
        t = tmp[:, :w]
        np.multiply(inT[:, lo + s:i * TLOC + TLOC], uinv, out=t)
        np.rint(t, out=t)
        ub = _CACHE["ubufs"][i]
        ub[:, s:] = t                       # cast-assign: exact for integers
        shards.append(jax.device_put(ub, exe["devs"][i]))
    return jax.make_array_from_single_device_arrays(
        (NCORE * DU, ULEN), exe["sh_core"], shards)


def kernel(data, inputs, mean, A, B, C, recognition_matrix, steps=None, **kw):
    data = np.asarray(data, np.float32)
    inputs_np = np.asarray(inputs, np.float32)
    mean = np.asarray(mean, np.float32)
    A = np.asarray(A, np.float32)
    B = np.asarray(B, np.float32)
    C = np.asarray(C, np.float32)
    R = np.asarray(recognition_matrix, np.float32)

    exe = _build()
    outz = exe["zmaker"]()                      # async, on-device zeros

    ucol = np.abs(inputs_np).max(axis=0)
    # constants are cached on device across calls keyed by content; any
    # change in A/B/C/mean/input scales recomputes and re-uploads
    kh = hashlib.blake2b(
        A.tobytes() + B.tobytes() + C.tobytes() + mean.tobytes()
        + ucol.tobytes(), digest_size=16).hexdigest()
    if _CACHE.get("kon_key") != kh:
        kon = _make_kon(mean, A, B, C, ucol)
        kon0 = jax.device_put(kon, exe["devs"][0])
        _CACHE["kon_rep"] = jax.device_put(kon0, exe["sh_rep"])
        _CACHE["kon_key"] = kh
    kon_rep = _CACHE["kon_rep"]

    u_dev = _stream_u(inputs_np, ucol, exe)
    try:
        (out_dev,) = exe["sharded"](u_dev, kon_rep, outz)
    except Exception:
        # one retry: a previously crashed process can leave the exec unit
        # wedged; the failed attempt resets it
        outz = exe["zmaker"]()
        (out_dev,) = exe["sharded"](u_dev, kon_rep, outz)

    # host correction while the result streams back: out row n-1 +=
    # (A^n z0) @ B.T.  ||A^n z0|| ~ 0.9^n, so 64 rows reach ~1e-3 of a
    # unit (well under the int8 quantization noise).
    HC = 64
    z0 = R @ (data[0] - mean[0])
    zc = z0
    corr = np.empty((HC, DZ), np.float32)
    for n in range(1, HC + 1):
        zc = A @ zc
        corr[n - 1] = B @ zc

    buf = np.asarray(out_dev)                   # blocks on D2H
    scale = buf[:, DZ:OW].copy().view(np.float32) * np.float32(1.0 / 127.0)
    out = np.empty((T, DZ), np.float32)
    np.multiply(buf[:, 0:DZ], scale, out=out)
    out[:HC] += corr
    return out


# revision 20
# speedup vs baseline: 1.1022x; 1.0210x over previous
"""Trainium2 Bass kernel for the KalmanFilter linear recurrence.

  x = data - mean;  z0 = R @ x[0];  drive = inputs @ C.T
  z_{t+1} = A z_t + drive[t]   (T = 32768 steps, dim 512)
  result  = Z[1:] @ B.T + mean

Strategy (8 NeuronCores, sequence-parallel, no collectives):
  - ||A^k|| decays like 0.9^k (spectral radius 0.9), so the recurrence
    forgets its state after H=128 steps to ~1e-5 relative.
  - Each core owns 4096 contiguous steps, split into 256 chunks of S=16
    steps + K=8 extra "halo" chunks covering the preceding H=128 steps.
  - Phase A: batched zero-init scan over all 264 chunks (state tiles
    [512, 264], 15 matmul steps) -> per-chunk accumulated drives b_c.
  - Phase B: chunk-start states w_c = sum_{p=0}^{K-1} (A^16)^p b_{c-1-p}
    (banded combine; truncated at ||A^128|| ~ 4e-4 of a unit).
  - Phase C: re-scan the 256 real chunks from inits w_c; each step also
    applies the output projection B.T (+mean) and streams rows to DRAM.
  - z0 only affects output rows 0..H-1 (through A^n z0); that correction
    is added on the host, so the device never sees `data`/`R`.

  Wall time is dominated by the host<->device tunnel (~55MB/s), so the
  wire format is aggressively compressed: inputs ship as int8 with
  per-feature scales folded into C.T on the host; outputs ship as int8
  rows with a per-row abs-max scale (f32) packed into 4 extra int8
  columns of the same tensor. Matrix constants ship fp16, packed into
  one tensor uploaded to core 0, broadcast device-to-device, and cached
  on device across calls keyed by a content hash. Donated output zero
  buffers are created on device. Matmuls run fp16 with f32 PSUM
  accumulation. The jit executable is built once and cached.
"""
import hashlib
import numpy as np
import jax
import jax.numpy as jnp
from jax.experimental.shard_map import shard_map
from jax.sharding import Mesh, NamedSharding, PartitionSpec as P

import concourse.bacc as bacc
import concourse.mybir as mybir
from concourse import tile
from concourse.bass2jax import (
    _bass_exec_p, install_neuronx_cc_hook, partition_id_tensor)

T = 32768
DZ = 512
DU = 256
NCORE = 8
TLOC = T // NCORE          # 4096
S = 16                     # steps per chunk
BCH = TLOC // S            # 256 chunks per core
H = 128                    # halo steps (forgetting horizon)
K = H // S                 # 8 banded taps (incl. identity)
NCH = BCH + K              # 264 chunks in phase A
ULEN = TLOC + H            # 4224 drive rows per core (multiple of 128)
OW = DZ + 4                # 516: int8 row + 4 bytes of f32 row scale
# packed constants: at(512) bt(512) ct(256) mn(128) mb(7*512)
KROWS = 512 + 512 + 256 + 128 + (K - 1) * 512   # 4992
MBOFF = 1408

f16 = mybir.dt.float16
f32 = mybir.dt.float32
i8 = mybir.dt.int8

_CACHE = {}


def _emit(nc):
    u_d = nc.dram_tensor("u", (2 * 128, ULEN), i8, kind="ExternalInput")
    kon_d = nc.dram_tensor("kon", (KROWS, DZ), f16, kind="ExternalInput")
    out_d = nc.dram_tensor("out", (TLOC, OW), i8, kind="ExternalOutput")

    with tile.TileContext(nc) as tc:
        with tc.tile_pool(name="const", bufs=1) as cpool, \
             tc.tile_pool(name="dt", bufs=1) as dpool, \
             tc.tile_pool(name="ut", bufs=1) as upool, \
             tc.tile_pool(name="mb", bufs=3) as mbpool, \
             tc.tile_pool(name="st", bufs=2) as stpool, \
             tc.tile_pool(name="ob", bufs=4) as opool, \
             tc.tile_pool(name="sc", bufs=8) as scpool, \
             tc.tile_pool(name="ps", bufs=8, space="PSUM") as pp:

            # ---- constant loads (packed rows of kon) ----
            at_sb = [cpool.tile([128, DZ], f16, tag=f"at{k}", name=f"at{k}") for k in range(4)]
            bt_sb = [cpool.tile([128, DZ], f16, tag=f"bt{k}", name=f"bt{k}") for k in range(4)]
            ct_sb = [cpool.tile([128, DZ], f16, tag=f"ct{k}", name=f"ct{k}") for k in range(2)]
            mn_sb = cpool.tile([128, DZ], f16, tag="mn")
            for k in range(4):
                nc.sync.dma_start(at_sb[k][:], kon_d[128 * k:128 * (k + 1), :])
            for k in range(4):
                nc.sync.dma_start(bt_sb[k][:], kon_d[512 + 128 * k:512 + 128 * (k + 1), :])
            for k in range(2):
                nc.sync.dma_start(ct_sb[k][:], kon_d[1024 + 128 * k:1024 + 128 * (k + 1), :])
            nc.sync.dma_start(mn_sb[:], kon_d[1280:1408, :])

            # u.T tiles (int8 on the wire, widened to fp16 for the PE)
            uq_sb = [upool.tile([128, ULEN], i8, tag=f"uq{k}", name=f"uq{k}") for k in range(2)]
            ut_sb = [upool.tile([128, ULEN], f16, tag=f"ut{k}", name=f"ut{k}") for k in range(2)]
            for k in range(2):
                nc.sync.dma_start(uq_sb[k][:], u_d[128 * k:128 * (k + 1), :])
            for k in range(2):
                nc.vector.tensor_copy(ut_sb[k][:], uq_sb[k][:])

            # drive rows (transposed): dt[m] holds drive.T[128m:128(m+1), :]
            dt_sb = [dpool.tile([128, ULEN], f16, tag=f"dt{m}", name=f"dt{m}") for m in range(4)]
            for nb in range((ULEN + 511) // 512):
                nb0 = nb * 512
                w = min(512, ULEN - nb0)
                for m in range(4):
                    psd = pp.tile([128, 512], f32, tag="ps")
                    for kk in range(2):
                        nc.tensor.matmul(
                            psd[:, :w],
                            ct_sb[kk][:, 128 * m:128 * (m + 1)],
                            ut_sb[kk][:, nb0:nb0 + w],
                            start=(kk == 0), stop=(kk == 1))
                    nc.any.tensor_copy(dt_sb[m][:, nb0:nb0 + w], psd[:, :w])

            # ---- phase A: zero-init scan over NCH chunks ----
            bmat = [cpool.tile([128, NCH], f16, tag=f"bm{m}", name=f"bm{m}") for m in range(4)]
            st_prev = []
            for m in range(4):
                t0 = stpool.tile([128, NCH], f16, tag=f"st{m}", name=f"st0_{m}")
                nc.vector.tensor_copy(t0[:], dt_sb[m][:, 0:16 * (NCH - 1) + 1:16])
                st_prev.append(t0)
            for k in range(1, S):
                psl = [pp.tile([128, NCH], f32, tag="ps", name=f"psA{k}_{_m}") for _m in range(4)]
                for m in range(4):
                    for kk in range(4):
                        nc.tensor.matmul(
                            psl[m][:],
                            at_sb[kk][:, 128 * m:128 * (m + 1)],
                            st_prev[kk][:],
                            start=(kk == 0), stop=(kk == 3))
                st_new = []
                for m in range(4):
                    dst = (bmat[m] if k == S - 1 else
                           stpool.tile([128, NCH], f16, tag=f"st{m}", name=f"stA{k}_{m}"))
                    nc.vector.tensor_tensor(
                        dst[:], psl[m][:],
                        dt_sb[m][:, k:k + 16 * (NCH - 1) + 1:16],
                        op=mybir.AluOpType.add)
                    st_new.append(dst)
                st_prev = st_new

            # ---- phase B: banded combine  w_c = sum_p M_p b_{c-1-p} ----
            psw = [pp.tile([128, BCH], f32, tag="ps", name=f"psW{_m}") for _m in range(4)]
            for p in range(1, K):
                mbt = mbpool.tile([128, 4 * DZ], f16, tag="mbt")
                off = MBOFF + (p - 1) * 512
                nc.sync.dma_start(
                    mbt[:].rearrange("p (k n) -> p k n", k=4),
                    kon_d[off:off + 512, :].rearrange("(p k) n -> p k n", k=4))
                lo = K - 1 - p
                for m in range(4):
                    for kk in range(4):
                        nc.tensor.matmul(
                            psw[m][:],
                            mbt[:, 512 * kk + 128 * m:512 * kk + 128 * m + 128],
                            bmat[kk][:, lo:lo + BCH],
                            start=(p == 1 and kk == 0),
                            stop=(p == K - 1 and kk == 3))
            w_sb = []
            for m in range(4):
                wt = cpool.tile([128, BCH], f16, tag=f"w{m}", name=f"w{m}")
                nc.vector.tensor_tensor(
                    wt[:], psw[m][:], bmat[m][:, K - 1:K - 1 + BCH],
                    op=mybir.AluOpType.add)
                w_sb.append(wt)

            # ---- phase C: scan 256 chunks from w_c, fused output proj ----
            st_prev = w_sb
            for k in range(S):
                psl = [pp.tile([128, BCH], f32, tag="ps", name=f"psC{k}_{_m}") for _m in range(4)]
                for m in range(4):
                    for kk in range(4):
                        nc.tensor.matmul(
                            psl[m][:],
                            at_sb[kk][:, 128 * m:128 * (m + 1)],
                            st_prev[kk][:],
                            start=(kk == 0), stop=(kk == 3))
                st_new = []
                for m in range(4):
                    dst = stpool.tile([128, BCH], f16, tag=f"sc{m}", name=f"stC{k}_{m}")
                    nc.vector.tensor_tensor(
                        dst[:], psl[m][:],
                        dt_sb[m][:, H + k:H + k + 16 * (BCH - 1) + 1:16],
                        op=mybir.AluOpType.add)
                    st_new.append(dst)
                st_prev = st_new
                # output rows t = 16*c + k, int8 with per-row abs-max scale
                # (HW f32->int8 conversion rounds-to-nearest and saturates;
                # CoreSim truncates/wraps, so sim overreports quant error)
                for h in range(BCH // 128):
                    pso = pp.tile([128, DZ], f32, tag="ps")
                    for kk in range(4):
                        nc.tensor.matmul(
                            pso[:],
                            st_new[kk][:, 128 * h:128 * (h + 1)],
                            bt_sb[kk][:],
                            start=(kk == 0), stop=(kk == 3))
                    obf = opool.tile([128, DZ], f32, tag="ob")
                    nc.vector.tensor_tensor(
                        obf[:], pso[:], mn_sb[:], op=mybir.AluOpType.add)
                    amax = scpool.tile([128, 1], f32, tag="am")
                    nc.vector.tensor_reduce(
                        amax[:], obf[:], axis=mybir.AxisListType.X,
                        op=mybir.AluOpType.max, apply_absolute_value=True)
                    inv = scpool.tile([128, 1], f32, tag="iv")
                    nc.vector.reciprocal(inv[:], amax[:])
                    qt = opool.tile([128, OW], i8, tag="qt")
                    nc.vector.tensor_scalar(
                        qt[:, 0:DZ], obf[:], inv[:], 127.0,
                        op0=mybir.AluOpType.mult, op1=mybir.AluOpType.mult)
                    # pack the f32 scale into the last 4 int8 columns
                    nc.vector.tensor_copy(
                        qt[:, DZ:OW].bitcast(f32), amax[:])
                    r0 = 2048 * h + k
                    nc.sync.dma_start(out_d[r0:r0 + 2033:16, :], qt[:])
    nc.compile()
    return nc


def _build():
    """Compile the bass module + jit executable once; reuse across calls."""
    if "exe" in _CACHE:
        return _CACHE["exe"]

    install_neuronx_cc_hook()
    nc = bacc.Bacc("TRN2", target_bir_lowering=False, debug=False)
    _emit(nc)

    # in/out names in BIR allocation order (mirrors run_bass_via_pjrt):
    # partition_id is excluded here and appended as the LAST operand,
    # supplied on-device by the PartitionIdOp primitive.
    part_name = nc.partition_id_tensor.name if nc.partition_id_tensor else None
    in_names, out_names, out_avals = [], [], []
    for alloc in nc.m.functions[0].allocations:
        if not isinstance(alloc, mybir.MemoryLocationSet):
            continue
        name = alloc.memorylocations[0].name
        if alloc.kind == "ExternalInput":
            if name != part_name:
                in_names.append(name)
        elif alloc.kind == "ExternalOutput":
            out_names.append(name)
            out_avals.append(jax.core.ShapedArray(
                tuple(alloc.tensor_shape), mybir.dt.np(alloc.dtype)))
    assert in_names == ["u", "kon"], in_names
    assert out_names == ["out"], out_names
    all_names = tuple(in_names) + tuple(out_names)
    if part_name is not None:
        all_names = all_names + (part_name,)

    devs = jax.devices()[:NCORE]
    mesh = Mesh(np.asarray(devs), ("core",))
    sh_core = NamedSharding(mesh, P("core"))
    sh_rep = NamedSharding(mesh, P())

    def _body(u, kon, outz):
        operands = [u, kon, outz]
        if part_name is not None:
            operands.append(partition_id_tensor())
        outs = _bass_exec_p.bind(
            *operands,
            out_avals=tuple(out_avals),
            in_names=all_names,
            out_names=tuple(out_names),
            lowering_input_output_aliases=(),
            sim_require_finite=True,
            sim_require_nnan=True,
            nc=nc)
        return tuple(outs)

    sharded = jax.jit(
        shard_map(_body, mesh=mesh,
                  in_specs=(P("core"), P(), P("core")),
                  out_specs=(P("core"),), check_rep=False),
        donate_argnums=(2,), keep_unused=True)
    zmaker = jax.jit(lambda: jnp.zeros((NCORE * TLOC, OW), jnp.int8),
                     out_shardings=sh_core)

    exe = {"sharded": sharded, "zmaker": zmaker, "devs": devs,
           "sh_core": sh_core, "sh_rep": sh_rep}
    _CACHE["exe"] = exe
    return exe


def _make_kon(mean, A, B, C, ucol):
    """Packed fp16 constants; u int8 scales are folded into C.T rows."""
    AS = np.linalg.matrix_power(A, S)
    kon = np.empty((KROWS, DZ), np.float16)
    kon[0:512] = A.T
    kon[512:1024] = B.T
    kon[1024:1280] = C.T * (ucol / np.float32(127.0))[:, None]
    kon[1280:1408] = np.broadcast_to(mean, (128, DZ))
    Mp = AS.copy()
    for p in range(1, K):
        off = MBOFF + (p - 1) * 512
        kon[off:off + 512] = (
            Mp.T.reshape(4, 128, DZ).transpose(1, 0, 2).reshape(512, DZ))
        Mp = Mp @ AS
    return kon


def _stream_u(inputs_np, ucol, exe):
    """Per-core int8 quantization streamed into per-device uploads, so
    the CPU quant of core i+1 overlaps the wire transfer of core i."""
    uinv = (np.float32(127.0) / ucol)[:, None]
    inT = inputs_np.T
    if "ubufs" not in _CACHE:
        # core 0's H-column halo stays zero across calls
        _CACHE["ubufs"] = [np.zeros((DU, ULEN), np.int8) for _ in range(NCORE)]
        _CACHE["utmp"] = np.empty((DU, ULEN), np.float32)
    tmp = _CACHE["utmp"]
    shards = []
    for i in range(NCORE):
        lo = i * TLOC - H
        s = max(0, -lo)
        w = ULEN - s
        t = tmp[:, :w]
        np.multiply(inT[:, lo + s:i * TLOC + TLOC], uinv, out=t)
        np.rint(t, out=t)
        ub = _CACHE["ubufs"][i]
        ub[:, s:] = t                       # cast-assign: exact for integers
        shards.append(jax.device_put(ub, exe["devs"][i]))
    return jax.make_array_from_single_device_arrays(
        (NCORE * DU, ULEN), exe["sh_core"], shards)


def kernel(data, inputs, mean, A, B, C, recognition_matrix, steps=None, **kw):
    data = np.asarray(data, np.float32)
    inputs_np = np.asarray(inputs, np.float32)
    mean = np.asarray(mean, np.float32)
    A = np.asarray(A, np.float32)
    B = np.asarray(B, np.float32)
    C = np.asarray(C, np.float32)
    R = np.asarray(recognition_matrix, np.float32)

    exe = _build()
    outz = exe["zmaker"]()                      # async, on-device zeros

    ucol = np.maximum(np.abs(inputs_np).max(axis=0), np.float32(1e-30))
    # constants are cached on device across calls keyed by content; any
    # change in A/B/C/mean/input scales recomputes and re-uploads
    kh = hashlib.blake2b(
        A.tobytes() + B.tobytes() + C.tobytes() + mean.tobytes()
        + ucol.tobytes(), digest_size=16).hexdigest()
    if _CACHE.get("kon_key") != kh:
        kon = _make_kon(mean, A, B, C, ucol)
        kon0 = jax.device_put(kon, exe["devs"][0])
        _CACHE["kon_rep"] = jax.device_put(kon0, exe["sh_rep"])
        _CACHE["kon_key"] = kh
    kon_rep = _CACHE["kon_rep"]

    u_dev = _stream_u(inputs_np, ucol, exe)
    try:
        (out_dev,) = exe["sharded"](u_dev, kon_rep, outz)
    except Exception:
        # one retry: a previously crashed process can leave the exec unit
        # wedged; the failed attempt resets it
        outz = exe["zmaker"]()
        (out_dev,) = exe["sharded"](u_dev, kon_rep, outz)

    # host correction while the result streams back: out row n-1 +=
    # (A^n z0) @ B.T.  ||A^n z0|| ~ 0.9^n, so 64 rows reach ~1e-3 of a
    # unit (well under the int8 quantization noise).
    HC = 64
    z0 = R @ (data[0] - mean[0])
    zc = z0
    corr = np.empty((HC, DZ), np.float32)
    for n in range(1, HC + 1):
        zc = A @ zc
        corr[n - 1] = B @ zc

    buf = np.asarray(out_dev)                   # blocks on D2H
    scale = buf[:, DZ:OW].copy().view(np.float32) * np.float32(1.0 / 127.0)
    out = np.empty((T, DZ), np.float32)
    np.multiply(buf[:, 0:DZ], scale, out=out)
    out[:HC] += corr
    return out


# revision 21
# speedup vs baseline: 1.1874x; 1.0773x over previous
"""Trainium2 Bass kernel for the KalmanFilter linear recurrence.

  x = data - mean;  z0 = R @ x[0];  drive = inputs @ C.T
  z_{t+1} = A z_t + drive[t]   (T = 32768 steps, dim 512)
  result  = Z[1:] @ B.T + mean

Strategy (8 NeuronCores, sequence-parallel, no collectives):
  - ||A^k|| decays like 0.9^k (spectral radius 0.9), so the recurrence
    forgets its state after H=128 steps to ~1e-5 relative.
  - Each core owns 4096 contiguous steps, split into 256 chunks of S=16
    steps + K=8 extra "halo" chunks covering the preceding H=128 steps.
  - Phase A: batched zero-init scan over all 264 chunks (state tiles
    [512, 264], 15 matmul steps) -> per-chunk accumulated drives b_c.
  - Phase B: chunk-start states w_c = sum_{p=0}^{K-1} (A^16)^p b_{c-1-p}
    (banded combine; truncated at ||A^128|| ~ 4e-4 of a unit).
  - Phase C: re-scan the 256 real chunks from inits w_c; each step also
    applies the output projection B.T (+mean) and streams rows to DRAM.
  - z0 only affects output rows 0..H-1 (through A^n z0); that correction
    is added on the host, so the device never sees `data`/`R`.

  Wall time is dominated by the host<->device tunnel (~55MB/s), so the
  wire format is aggressively compressed (vs ~220MB/call for the naive
  f32 layout): inputs ship as int8 (8.7MB) with per-feature scales
  folded into C.T on the host, and the per-core quantization is
  streamed so CPU quant overlaps the upload; outputs ship as int8 rows
  (16.9MB) with a per-row abs-max scale (f32) packed into 4 extra int8
  columns of the same tensor. Matrix constants ship fp16 packed in one
  tensor (4.9MB) uploaded to core 0, broadcast device-to-device, and
  cached on device across calls keyed by a content hash. Donated output
  zero buffers are created on device. Matmuls run fp16 with f32 PSUM
  accumulation (end-to-end relfro ~1.1e-2 vs the 2e-2 gate). The jit
  executable is built once and cached across calls.
"""
import hashlib
import numpy as np
import jax
import jax.numpy as jnp
from jax.experimental.shard_map import shard_map
from jax.sharding import Mesh, NamedSharding, PartitionSpec as P

import concourse.bacc as bacc
import concourse.mybir as mybir
from concourse import tile
from concourse.bass2jax import (
    _bass_exec_p, install_neuronx_cc_hook, partition_id_tensor)

T = 32768
DZ = 512
DU = 256
NCORE = 8
TLOC = T // NCORE          # 4096
S = 16                     # steps per chunk
BCH = TLOC // S            # 256 chunks per core
H = 128                    # halo steps (forgetting horizon)
K = H // S                 # 8 banded taps (incl. identity)
NCH = BCH + K              # 264 chunks in phase A
ULEN = TLOC + H            # 4224 drive rows per core (multiple of 128)
OW = DZ + 4                # 516: int8 row + 4 bytes of f32 row scale
# packed constants: at(512) bt(512) ct(256) mn(128) mb(7*512)
KROWS = 512 + 512 + 256 + 128 + (K - 1) * 512   # 4992
MBOFF = 1408

f16 = mybir.dt.float16
f32 = mybir.dt.float32
i8 = mybir.dt.int8

_CACHE = {}


def _emit(nc):
    u_d = nc.dram_tensor("u", (2 * 128, ULEN), i8, kind="ExternalInput")
    kon_d = nc.dram_tensor("kon", (KROWS, DZ), f16, kind="ExternalInput")
    out_d = nc.dram_tensor("out", (TLOC, OW), i8, kind="ExternalOutput")

    with tile.TileContext(nc) as tc:
        with tc.tile_pool(name="const", bufs=1) as cpool, \
             tc.tile_pool(name="dt", bufs=1) as dpool, \
             tc.tile_pool(name="ut", bufs=1) as upool, \
             tc.tile_pool(name="mb", bufs=3) as mbpool, \
             tc.tile_pool(name="st", bufs=2) as stpool, \
             tc.tile_pool(name="ob", bufs=4) as opool, \
             tc.tile_pool(name="sc", bufs=8) as scpool, \
             tc.tile_pool(name="ps", bufs=8, space="PSUM") as pp:

            # ---- constant loads (packed rows of kon) ----
            at_sb = [cpool.tile([128, DZ], f16, tag=f"at{k}", name=f"at{k}") for k in range(4)]
            bt_sb = [cpool.tile([128, DZ], f16, tag=f"bt{k}", name=f"bt{k}") for k in range(4)]
            ct_sb = [cpool.tile([128, DZ], f16, tag=f"ct{k}", name=f"ct{k}") for k in range(2)]
            mn_sb = cpool.tile([128, DZ], f16, tag="mn")
            for k in range(4):
                nc.sync.dma_start(at_sb[k][:], kon_d[128 * k:128 * (k + 1), :])
            for k in range(4):
                nc.sync.dma_start(bt_sb[k][:], kon_d[512 + 128 * k:512 + 128 * (k + 1), :])
            for k in range(2):
                nc.sync.dma_start(ct_sb[k][:], kon_d[1024 + 128 * k:1024 + 128 * (k + 1), :])
            nc.sync.dma_start(mn_sb[:], kon_d[1280:1408, :])

            # u.T tiles (int8 on the wire, widened to fp16 for the PE)
            uq_sb = [upool.tile([128, ULEN], i8, tag=f"uq{k}", name=f"uq{k}") for k in range(2)]
            ut_sb = [upool.tile([128, ULEN], f16, tag=f"ut{k}", name=f"ut{k}") for k in range(2)]
            for k in range(2):
                nc.sync.dma_start(uq_sb[k][:], u_d[128 * k:128 * (k + 1), :])
            for k in range(2):
                nc.vector.tensor_copy(ut_sb[k][:], uq_sb[k][:])

            # drive rows (transposed): dt[m] holds drive.T[128m:128(m+1), :]
            dt_sb = [dpool.tile([128, ULEN], f16, tag=f"dt{m}", name=f"dt{m}") for m in range(4)]
            for nb in range((ULEN + 511) // 512):
                nb0 = nb * 512
                w = min(512, ULEN - nb0)
                for m in range(4):
                    psd = pp.tile([128, 512], f32, tag="ps")
                    for kk in range(2):
                        nc.tensor.matmul(
                            psd[:, :w],
                            ct_sb[kk][:, 128 * m:128 * (m + 1)],
                            ut_sb[kk][:, nb0:nb0 + w],
                            start=(kk == 0), stop=(kk == 1))
                    nc.any.tensor_copy(dt_sb[m][:, nb0:nb0 + w], psd[:, :w])

            # ---- phase A: zero-init scan over NCH chunks ----
            bmat = [cpool.tile([128, NCH], f16, tag=f"bm{m}", name=f"bm{m}") for m in range(4)]
            st_prev = []
            for m in range(4):
                t0 = stpool.tile([128, NCH], f16, tag=f"st{m}", name=f"st0_{m}")
                nc.vector.tensor_copy(t0[:], dt_sb[m][:, 0:16 * (NCH - 1) + 1:16])
                st_prev.append(t0)
            for k in range(1, S):
                psl = [pp.tile([128, NCH], f32, tag="ps", name=f"psA{k}_{_m}") for _m in range(4)]
                for m in range(4):
                    for kk in range(4):
                        nc.tensor.matmul(
                            psl[m][:],
                            at_sb[kk][:, 128 * m:128 * (m + 1)],
                            st_prev[kk][:],
                            start=(kk == 0), stop=(kk == 3))
                st_new = []
                for m in range(4):
                    dst = (bmat[m] if k == S - 1 else
                           stpool.tile([128, NCH], f16, tag=f"st{m}", name=f"stA{k}_{m}"))
                    nc.vector.tensor_tensor(
                        dst[:], psl[m][:],
                        dt_sb[m][:, k:k + 16 * (NCH - 1) + 1:16],
                        op=mybir.AluOpType.add)
                    st_new.append(dst)
                st_prev = st_new

            # ---- phase B: banded combine  w_c = sum_p M_p b_{c-1-p} ----
            psw = [pp.tile([128, BCH], f32, tag="ps", name=f"psW{_m}") for _m in range(4)]
            for p in range(1, K):
                mbt = mbpool.tile([128, 4 * DZ], f16, tag="mbt")
                off = MBOFF + (p - 1) * 512
                nc.sync.dma_start(
                    mbt[:].rearrange("p (k n) -> p k n", k=4),
                    kon_d[off:off + 512, :].rearrange("(p k) n -> p k n", k=4))
                lo = K - 1 - p
                for m in range(4):
                    for kk in range(4):
                        nc.tensor.matmul(
                            psw[m][:],
                            mbt[:, 512 * kk + 128 * m:512 * kk + 128 * m + 128],
                            bmat[kk][:, lo:lo + BCH],
                            start=(p == 1 and kk == 0),
                            stop=(p == K - 1 and kk == 3))
            w_sb = []
            for m in range(4):
                wt = cpool.tile([128, BCH], f16, tag=f"w{m}", name=f"w{m}")
                nc.vector.tensor_tensor(
                    wt[:], psw[m][:], bmat[m][:, K - 1:K - 1 + BCH],
                    op=mybir.AluOpType.add)
                w_sb.append(wt)

            # ---- phase C: scan 256 chunks from w_c, fused output proj ----
            st_prev = w_sb
            for k in range(S):
                psl = [pp.tile([128, BCH], f32, tag="ps", name=f"psC{k}_{_m}") for _m in range(4)]
                for m in range(4):
                    for kk in range(4):
                        nc.tensor.matmul(
                            psl[m][:],
                            at_sb[kk][:, 128 * m:128 * (m + 1)],
                            st_prev[kk][:],
                            start=(kk == 0), stop=(kk == 3))
                st_new = []
                for m in range(4):
                    dst = stpool.tile([128, BCH], f16, tag=f"sc{m}", name=f"stC{k}_{m}")
                    nc.vector.tensor_tensor(
                        dst[:], psl[m][:],
                        dt_sb[m][:, H + k:H + k + 16 * (BCH - 1) + 1:16],
                        op=mybir.AluOpType.add)
                    st_new.append(dst)
                st_prev = st_new
                # output rows t = 16*c + k, int8 with per-row abs-max scale
                # (HW f32->int8 conversion rounds-to-nearest and saturates;
                # CoreSim truncates/wraps, so sim overreports quant error)
                for h in range(BCH // 128):
                    pso = pp.tile([128, DZ], f32, tag="ps")
                    for kk in range(4):
                        nc.tensor.matmul(
                            pso[:],
                            st_new[kk][:, 128 * h:128 * (h + 1)],
                            bt_sb[kk][:],
                            start=(kk == 0), stop=(kk == 3))
                    obf = opool.tile([128, DZ], f32, tag="ob")
                    nc.vector.tensor_tensor(
                        obf[:], pso[:], mn_sb[:], op=mybir.AluOpType.add)
                    amax = scpool.tile([128, 1], f32, tag="am")
                    nc.vector.tensor_reduce(
                        amax[:], obf[:], axis=mybir.AxisListType.X,
                        op=mybir.AluOpType.max, apply_absolute_value=True)
                    inv = scpool.tile([128, 1], f32, tag="iv")
                    nc.vector.reciprocal(inv[:], amax[:])
                    qt = opool.tile([128, OW], i8, tag="qt")
                    nc.vector.tensor_scalar(
                        qt[:, 0:DZ], obf[:], inv[:], 127.0,
                        op0=mybir.AluOpType.mult, op1=mybir.AluOpType.mult)
                    # pack the f32 scale into the last 4 int8 columns
                    nc.vector.tensor_copy(
                        qt[:, DZ:OW].bitcast(f32), amax[:])
                    r0 = 2048 * h + k
                    nc.sync.dma_start(out_d[r0:r0 + 2033:16, :], qt[:])
    nc.compile()
    return nc


def _build():
    """Compile the bass module + jit executable once; reuse across calls."""
    if "exe" in _CACHE:
        return _CACHE["exe"]

    install_neuronx_cc_hook()
    nc = bacc.Bacc("TRN2", target_bir_lowering=False, debug=False)
    _emit(nc)

    # in/out names in BIR allocation order (mirrors run_bass_via_pjrt):
    # partition_id is excluded here and appended as the LAST operand,
    # supplied on-device by the PartitionIdOp primitive.
    part_name = nc.partition_id_tensor.name if nc.partition_id_tensor else None
    in_names, out_names, out_avals = [], [], []
    for alloc in nc.m.functions[0].allocations:
        if not isinstance(alloc, mybir.MemoryLocationSet):
            continue
        name = alloc.memorylocations[0].name
        if alloc.kind == "ExternalInput":
            if name != part_name:
                in_names.append(name)
        elif alloc.kind == "ExternalOutput":
            out_names.append(name)
            out_avals.append(jax.core.ShapedArray(
                tuple(alloc.tensor_shape), mybir.dt.np(alloc.dtype)))
    assert in_names == ["u", "kon"], in_names
    assert out_names == ["out"], out_names
    all_names = tuple(in_names) + tuple(out_names)
    if part_name is not None:
        all_names = all_names + (part_name,)

    devs = jax.devices()[:NCORE]
    mesh = Mesh(np.asarray(devs), ("core",))
    sh_core = NamedSharding(mesh, P("core"))
    sh_rep = NamedSharding(mesh, P())

    def _body(u, kon, outz):
        operands = [u, kon, outz]
        if part_name is not None:
            operands.append(partition_id_tensor())
        outs = _bass_exec_p.bind(
            *operands,
            out_avals=tuple(out_avals),
            in_names=all_names,
            out_names=tuple(out_names),
            lowering_input_output_aliases=(),
            sim_require_finite=True,
            sim_require_nnan=True,
            nc=nc)
        return tuple(outs)

    sharded = jax.jit(
        shard_map(_body, mesh=mesh,
                  in_specs=(P("core"), P(), P("core")),
                  out_specs=(P("core"),), check_rep=False),
        donate_argnums=(2,), keep_unused=True)
    zmaker = jax.jit(lambda: jnp.zeros((NCORE * TLOC, OW), jnp.int8),
                     out_shardings=sh_core)

    exe = {"sharded": sharded, "zmaker": zmaker, "devs": devs,
           "sh_core": sh_core, "sh_rep": sh_rep}
    _CACHE["exe"] = exe
    return exe


def _make_kon(mean, A, B, C, ucol):
    """Packed fp16 constants; u int8 scales are folded into C.T rows."""
    AS = np.linalg.matrix_power(A, S)
    kon = np.empty((KROWS, DZ), np.float16)
    kon[0:512] = A.T
    kon[512:1024] = B.T
    kon[1024:1280] = C.T * (ucol / np.float32(127.0))[:, None]
    kon[1280:1408] = np.broadcast_to(mean, (128, DZ))
    Mp = AS.copy()
    for p in range(1, K):
        off = MBOFF + (p - 1) * 512
        kon[off:off + 512] = (
            Mp.T.reshape(4, 128, DZ).transpose(1, 0, 2).reshape(512, DZ))
        Mp = Mp @ AS
    return kon


def _stream_u(inputs_np, ucol, exe):
    """Per-core int8 quantization streamed into per-device uploads, so
    the CPU quant of core i+1 overlaps the wire transfer of core i."""
    uinv = (np.float32(127.0) / ucol)[:, None]
    inT = inputs_np.T
    if "ubufs" not in _CACHE:
        # core 0's H-column halo stays zero across calls
        _CACHE["ubufs"] = [np.zeros((DU, ULEN), np.int8) for _ in range(NCORE)]
        _CACHE["utmp"] = np.empty((DU, ULEN), np.float32)
    tmp = _CACHE["utmp"]
    shards = []
    for i in range(NCORE):
        lo = i * TLOC - H
        s = max(0, -lo)
        w = ULEN - s
        t = tmp[:, :w]
        np.multiply(inT[:, lo + s:i * TLOC + TLOC], uinv, out=t)
        np.rint(t, out=t)
        ub = _CACHE["ubufs"][i]
        ub[:, s:] = t                       # cast-assign: exact for integers
        shards.append(jax.device_put(ub, exe["devs"][i]))
    return jax.make_array_from_single_device_arrays(
        (NCORE * DU, ULEN), exe["sh_core"], shards)


def kernel(data, inputs, mean, A, B, C, recognition_matrix, steps=None, **kw):
    data = np.asarray(data, np.float32)
    inputs_np = np.asarray(inputs, np.float32)
    mean = np.asarray(mean, np.float32)
    A = np.asarray(A, np.float32)
    B = np.asarray(B, np.float32)
    C = np.asarray(C, np.float32)
    R = np.asarray(recognition_matrix, np.float32)

    exe = _build()
    outz = exe["zmaker"]()                      # async, on-device zeros

    ucol = np.maximum(np.abs(inputs_np).max(axis=0), np.float32(1e-30))
    # constants are cached on device across calls keyed by content; any
    # change in A/B/C/mean/input scales recomputes and re-uploads
    kh = hashlib.blake2b(
        A.tobytes() + B.tobytes() + C.tobytes() + mean.tobytes()
        + ucol.tobytes(), digest_size=16).hexdigest()
    if _CACHE.get("kon_key") != kh:
        kon = _make_kon(mean, A, B, C, ucol)
        kon0 = jax.device_put(kon, exe["devs"][0])
        _CACHE["kon_rep"] = jax.device_put(kon0, exe["sh_rep"])
        _CACHE["kon_key"] = kh
    kon_rep = _CACHE["kon_rep"]

    u_dev = _stream_u(inputs_np, ucol, exe)
    try:
        (out_dev,) = exe["sharded"](u_dev, kon_rep, outz)
    except Exception:
        # one retry: a previously crashed process can leave the exec unit
        # wedged; the failed attempt resets it
        outz = exe["zmaker"]()
        (out_dev,) = exe["sharded"](u_dev, kon_rep, outz)

    # host correction while the result streams back: out row n-1 +=
    # (A^n z0) @ B.T.  ||A^n z0|| ~ 0.9^n, so 64 rows reach ~1e-3 of a
    # unit (well under the int8 quantization noise).
    HC = 64
    z0 = R @ (data[0] - mean[0])
    zc = z0
    corr = np.empty((HC, DZ), np.float32)
    for n in range(1, HC + 1):
        zc = A @ zc
        corr[n - 1] = B @ zc

    buf = np.asarray(out_dev)                   # blocks on D2H
    scale = buf[:, DZ:OW].copy().view(np.float32) * np.float32(1.0 / 127.0)
    out = np.empty((T, DZ), np.float32)
    np.multiply(buf[:, 0:DZ], scale, out=out)
    out[:HC] += corr
    return out


# revision 26
# speedup vs baseline: 1.2517x; 1.0541x over previous
"""Trainium2 Bass kernel for the KalmanFilter linear recurrence.

  x = data - mean;  z0 = R @ x[0];  drive = inputs @ C.T
  z_{t+1} = A z_t + drive[t]   (T = 32768 steps, dim 512)
  result  = Z[1:] @ B.T + mean

Strategy (8 NeuronCores, sequence-parallel, no collectives):
  - ||A^k|| decays like 0.9^k (spectral radius 0.9), so the recurrence
    forgets its state after H=128 steps to ~1e-5 relative.
  - Each core owns 4096 contiguous steps, split into 256 chunks of S=16
    steps + K=8 extra "halo" chunks covering the preceding H=128 steps.
  - Phase A: batched zero-init scan over all 264 chunks (state tiles
    [512, 264], 15 matmul steps) -> per-chunk accumulated drives b_c.
  - Phase B: chunk-start states w_c = sum_{p=0}^{K-1} (A^16)^p b_{c-1-p}
    (banded combine; truncated at ||A^128|| ~ 4e-4 of a unit).
  - Phase C: re-scan the 256 real chunks from inits w_c; each step also
    applies the output projection B.T (+mean) and streams rows to DRAM.
  - z0 only affects output rows 0..H-1 (through A^n z0); that correction
    is added on the host, so the device never sees `data`/`R`.

  Wall time is dominated by the host<->device tunnel (~55MB/s), so the
  wire format is aggressively compressed (vs ~220MB/call for the naive
  f32 layout): inputs ship as int8 (8.7MB) with per-feature scales
  folded into C.T on the host, and the per-core quantization is
  streamed so CPU quant overlaps the upload; outputs ship as int8 rows
  (16.9MB) with a per-row abs-max scale (f32) packed into 4 extra int8
  columns of the same tensor. Matrix constants ship fp16 packed in one
  tensor (4.9MB) uploaded to core 0, broadcast device-to-device, and
  cached on device across calls keyed by a content hash. Donated output
  zero buffers are created on device. Matmuls run fp16 with f32 PSUM
  accumulation (end-to-end relfro ~1.1e-2 vs the 2e-2 gate). The jit
  executable is built once and cached across calls.
"""
import hashlib
import numpy as np
import jax
import jax.numpy as jnp
from jax.experimental.shard_map import shard_map
from jax.sharding import Mesh, NamedSharding, PartitionSpec as P

import concourse.bacc as bacc
import concourse.mybir as mybir
from concourse import tile
from concourse.bass2jax import (
    _bass_exec_p, install_neuronx_cc_hook, partition_id_tensor)

T = 32768
DZ = 512
DU = 256
NCORE = 8
TLOC = T // NCORE          # 4096 steps per core
NSTAGE = 2                 # pipeline stages per call (hides exec/dequant
                           # under the half-duplex tunnel transfers)
TLOC_S = TLOC // NSTAGE    # 2048 steps per core per stage
S = 16                     # steps per chunk
BCH = TLOC_S // S          # 128 chunks per core per stage
H = 128                    # halo steps (forgetting horizon)
K = H // S                 # 8 banded taps (incl. identity)
NCH = BCH + K              # 136 chunks in phase A
ULEN = TLOC_S + H          # 2176 drive rows per core per stage
OW = DZ + 4                # 516: int8 row + 4 bytes of f32 row scale
# packed constants: at(512) bt(512) ct(256) mn(128) mb(7*512)
KROWS = 512 + 512 + 256 + 128 + (K - 1) * 512   # 4992
MBOFF = 1408

f16 = mybir.dt.float16
f32 = mybir.dt.float32
i8 = mybir.dt.int8

_CACHE = {}


def _emit(nc):
    u_d = nc.dram_tensor("u", (2 * 128, ULEN), i8, kind="ExternalInput")
    kon_d = nc.dram_tensor("kon", (KROWS, DZ), f16, kind="ExternalInput")
    out_d = nc.dram_tensor("out", (TLOC_S, OW), i8, kind="ExternalOutput")

    with tile.TileContext(nc) as tc:
        with tc.tile_pool(name="const", bufs=1) as cpool, \
             tc.tile_pool(name="dt", bufs=1) as dpool, \
             tc.tile_pool(name="ut", bufs=1) as upool, \
             tc.tile_pool(name="mb", bufs=3) as mbpool, \
             tc.tile_pool(name="st", bufs=2) as stpool, \
             tc.tile_pool(name="ob", bufs=4) as opool, \
             tc.tile_pool(name="sc", bufs=8) as scpool, \
             tc.tile_pool(name="ps", bufs=8, space="PSUM") as pp:

            # ---- constant loads (packed rows of kon) ----
            at_sb = [cpool.tile([128, DZ], f16, tag=f"at{k}", name=f"at{k}") for k in range(4)]
            bt_sb = [cpool.tile([128, DZ], f16, tag=f"bt{k}", name=f"bt{k}") for k in range(4)]
            ct_sb = [cpool.tile([128, DZ], f16, tag=f"ct{k}", name=f"ct{k}") for k in range(2)]
            mn_sb = cpool.tile([128, DZ], f16, tag="mn")
            for k in range(4):
                nc.sync.dma_start(at_sb[k][:], kon_d[128 * k:128 * (k + 1), :])
            for k in range(4):
                nc.sync.dma_start(bt_sb[k][:], kon_d[512 + 128 * k:512 + 128 * (k + 1), :])
            for k in range(2):
                nc.sync.dma_start(ct_sb[k][:], kon_d[1024 + 128 * k:1024 + 128 * (k + 1), :])
            nc.sync.dma_start(mn_sb[:], kon_d[1280:1408, :])

            # u.T tiles (int8 on the wire, widened to fp16 for the PE)
            uq_sb = [upool.tile([128, ULEN], i8, tag=f"uq{k}", name=f"uq{k}") for k in range(2)]
            ut_sb = [upool.tile([128, ULEN], f16, tag=f"ut{k}", name=f"ut{k}") for k in range(2)]
            for k in range(2):
                nc.sync.dma_start(uq_sb[k][:], u_d[128 * k:128 * (k + 1), :])
            for k in range(2):
                nc.vector.tensor_copy(ut_sb[k][:], uq_sb[k][:])

            # drive rows (transposed): dt[m] holds drive.T[128m:128(m+1), :]
            dt_sb = [dpool.tile([128, ULEN], f16, tag=f"dt{m}", name=f"dt{m}") for m in range(4)]
            for nb in range((ULEN + 511) // 512):
                nb0 = nb * 512
                w = min(512, ULEN - nb0)
                for m in range(4):
                    psd = pp.tile([128, 512], f32, tag="ps")
                    for kk in range(2):
                        nc.tensor.matmul(
                            psd[:, :w],
                            ct_sb[kk][:, 128 * m:128 * (m + 1)],
                            ut_sb[kk][:, nb0:nb0 + w],
                            start=(kk == 0), stop=(kk == 1))
                    nc.any.tensor_copy(dt_sb[m][:, nb0:nb0 + w], psd[:, :w])

            # ---- phase A: zero-init scan over NCH chunks ----
            bmat = [cpool.tile([128, NCH], f16, tag=f"bm{m}", name=f"bm{m}") for m in range(4)]
            st_prev = []
            for m in range(4):
                t0 = stpool.tile([128, NCH], f16, tag=f"st{m}", name=f"st0_{m}")
                nc.vector.tensor_copy(t0[:], dt_sb[m][:, 0:16 * (NCH - 1) + 1:16])
                st_prev.append(t0)
            for k in range(1, S):
                psl = [pp.tile([128, NCH], f32, tag="ps", name=f"psA{k}_{_m}") for _m in range(4)]
                for m in range(4):
                    for kk in range(4):
                        nc.tensor.matmul(
                            psl[m][:],
                            at_sb[kk][:, 128 * m:128 * (m + 1)],
                            st_prev[kk][:],
                            start=(kk == 0), stop=(kk == 3))
                st_new = []
                for m in range(4):
                    dst = (bmat[m] if k == S - 1 else
                           stpool.tile([128, NCH], f16, tag=f"st{m}", name=f"stA{k}_{m}"))
                    nc.vector.tensor_tensor(
                        dst[:], psl[m][:],
                        dt_sb[m][:, k:k + 16 * (NCH - 1) + 1:16],
                        op=mybir.AluOpType.add)
                    st_new.append(dst)
                st_prev = st_new

            # ---- phase B: banded combine  w_c = sum_p M_p b_{c-1-p} ----
            psw = [pp.tile([128, BCH], f32, tag="ps", name=f"psW{_m}") for _m in range(4)]
            for p in range(1, K):
                mbt = mbpool.tile([128, 4 * DZ], f16, tag="mbt")
                off = MBOFF + (p - 1) * 512
                nc.sync.dma_start(
                    mbt[:].rearrange("p (k n) -> p k n", k=4),
                    kon_d[off:off + 512, :].rearrange("(p k) n -> p k n", k=4))
                lo = K - 1 - p
                for m in range(4):
                    for kk in range(4):
                        nc.tensor.matmul(
                            psw[m][:],
                            mbt[:, 512 * kk + 128 * m:512 * kk + 128 * m + 128],
                            bmat[kk][:, lo:lo + BCH],
                            start=(p == 1 and kk == 0),
                            stop=(p == K - 1 and kk == 3))
            w_sb = []
            for m in range(4):
                wt = cpool.tile([128, BCH], f16, tag=f"w{m}", name=f"w{m}")
                nc.vector.tensor_tensor(
                    wt[:], psw[m][:], bmat[m][:, K - 1:K - 1 + BCH],
                    op=mybir.AluOpType.add)
                w_sb.append(wt)

            # ---- phase C: scan 256 chunks from w_c, fused output proj ----
            st_prev = w_sb
            for k in range(S):
                psl = [pp.tile([128, BCH], f32, tag="ps", name=f"psC{k}_{_m}") for _m in range(4)]
                for m in range(4):
                    for kk in range(4):
                        nc.tensor.matmul(
                            psl[m][:],
                            at_sb[kk][:, 128 * m:128 * (m + 1)],
                            st_prev[kk][:],
                            start=(kk == 0), stop=(kk == 3))
                st_new = []
                for m in range(4):
                    dst = stpool.tile([128, BCH], f16, tag=f"sc{m}", name=f"stC{k}_{m}")
                    nc.vector.tensor_tensor(
                        dst[:], psl[m][:],
                        dt_sb[m][:, H + k:H + k + 16 * (BCH - 1) + 1:16],
                        op=mybir.AluOpType.add)
                    st_new.append(dst)
                st_prev = st_new
                # output rows t = 16*c + k, int8 with per-row abs-max scale
                # (HW f32->int8 conversion rounds-to-nearest and saturates;
                # CoreSim truncates/wraps, so sim overreports quant error)
                for h in range(BCH // 128):
                    pso = pp.tile([128, DZ], f32, tag="ps")
                    for kk in range(4):
                        nc.tensor.matmul(
                            pso[:],
                            st_new[kk][:, 128 * h:128 * (h + 1)],
                            bt_sb[kk][:],
                            start=(kk == 0), stop=(kk == 3))
                    obf = opool.tile([128, DZ], f32, tag="ob")
                    nc.vector.tensor_tensor(
                        obf[:], pso[:], mn_sb[:], op=mybir.AluOpType.add)
                    amax = scpool.tile([128, 1], f32, tag="am")
                    nc.vector.tensor_reduce(
                        amax[:], obf[:], axis=mybir.AxisListType.X,
                        op=mybir.AluOpType.max, apply_absolute_value=True)
                    inv = scpool.tile([128, 1], f32, tag="iv")
                    nc.vector.reciprocal(inv[:], amax[:])
                    qt = opool.tile([128, OW], i8, tag="qt")
                    nc.vector.tensor_scalar(
                        qt[:, 0:DZ], obf[:], inv[:], 127.0,
                        op0=mybir.AluOpType.mult, op1=mybir.AluOpType.mult)
                    # pack the f32 scale into the last 4 int8 columns
                    nc.vector.tensor_copy(
                        qt[:, DZ:OW].bitcast(f32), amax[:])
                    r0 = 2048 * h + k
                    nc.sync.dma_start(out_d[r0:r0 + 2033:16, :], qt[:])
    nc.compile()
    return nc


def _build():
    """Compile the bass module + jit executable once; reuse across calls."""
    if "exe" in _CACHE:
        return _CACHE["exe"]

    install_neuronx_cc_hook()
    nc = bacc.Bacc("TRN2", target_bir_lowering=False, debug=False)
    _emit(nc)

    # in/out names in BIR allocation order (mirrors run_bass_via_pjrt):
    # partition_id is excluded here and appended as the LAST operand,
    # supplied on-device by the PartitionIdOp primitive.
    part_name = nc.partition_id_tensor.name if nc.partition_id_tensor else None
    in_names, out_names, out_avals = [], [], []
    for alloc in nc.m.functions[0].allocations:
        if not isinstance(alloc, mybir.MemoryLocationSet):
            continue
        name = alloc.memorylocations[0].name
        if alloc.kind == "ExternalInput":
            if name != part_name:
                in_names.append(name)
        elif alloc.kind == "ExternalOutput":
            out_names.append(name)
            out_avals.append(jax.core.ShapedArray(
                tuple(alloc.tensor_shape), mybir.dt.np(alloc.dtype)))
    assert in_names == ["u", "kon"], in_names
    assert out_names == ["out"], out_names
    all_names = tuple(in_names) + tuple(out_names)
    if part_name is not None:
        all_names = all_names + (part_name,)

    devs = jax.devices()[:NCORE]
    mesh = Mesh(np.asarray(devs), ("core",))
    sh_core = NamedSharding(mesh, P("core"))
    sh_rep = NamedSharding(mesh, P())

    def _body(u, kon, outz):
        operands = [u, kon, outz]
        if part_name is not None:
            operands.append(partition_id_tensor())
        outs = _bass_exec_p.bind(
            *operands,
            out_avals=tuple(out_avals),
            in_names=all_names,
            out_names=tuple(out_names),
            lowering_input_output_aliases=(),
            sim_require_finite=True,
            sim_require_nnan=True,
            nc=nc)
        return tuple(outs)

    sharded = jax.jit(
        shard_map(_body, mesh=mesh,
                  in_specs=(P("core"), P(), P("core")),
                  out_specs=(P("core"),), check_rep=False),
        donate_argnums=(2,), keep_unused=True)
    zmaker = jax.jit(lambda: jnp.zeros((NCORE * TLOC_S, OW), jnp.int8),
                     out_shardings=sh_core)

    exe = {"sharded": sharded, "zmaker": zmaker, "devs": devs,
           "sh_core": sh_core, "sh_rep": sh_rep}
    _CACHE["exe"] = exe
    return exe


def _make_kon(mean, A, B, C, ucol):
    """Packed fp16 constants; u int8 scales are folded into C.T rows."""
    AS = np.linalg.matrix_power(A, S)
    kon = np.empty((KROWS, DZ), np.float16)
    kon[0:512] = A.T
    kon[512:1024] = B.T
    kon[1024:1280] = C.T * (ucol / np.float32(127.0))[:, None]
    kon[1280:1408] = np.broadcast_to(mean, (128, DZ))
    Mp = AS.copy()
    for p in range(1, K):
        off = MBOFF + (p - 1) * 512
        kon[off:off + 512] = (
            Mp.T.reshape(4, 128, DZ).transpose(1, 0, 2).reshape(512, DZ))
        Mp = Mp @ AS
    return kon


def _stream_u(inputs_np, uinv, stage, exe):
    """Per-core int8 quantization of one pipeline stage, streamed into
    per-device uploads so the CPU quant of core i+1 overlaps the wire
    transfer of core i (and stage 1's quant overlaps stage 0's exec)."""
    inT = inputs_np.T
    if "ubufs" not in _CACHE:
        # stage0/core0's H-column halo stays zero across calls
        _CACHE["ubufs"] = [
            [np.zeros((DU, ULEN), np.int8) for _ in range(NCORE)]
            for _ in range(NSTAGE)]
        _CACHE["utmp"] = np.empty((DU, ULEN), np.float32)
    tmp = _CACHE["utmp"]
    shards = []
    for i in range(NCORE):
        base = i * TLOC + stage * TLOC_S
        lo = base - H
        s = max(0, -lo)
        t = tmp[:, :ULEN - s]
        np.multiply(inT[:, lo + s:base + TLOC_S], uinv, out=t)
        np.rint(t, out=t)
        ub = _CACHE["ubufs"][stage][i]
        ub[:, s:] = t                       # cast-assign: exact for integers
        shards.append(jax.device_put(ub, exe["devs"][i]))
    return jax.make_array_from_single_device_arrays(
        (NCORE * DU, ULEN), exe["sh_core"], shards)


def kernel(data, inputs, mean, A, B, C, recognition_matrix, steps=None, **kw):
    data = np.asarray(data, np.float32)
    inputs_np = np.asarray(inputs, np.float32)
    mean = np.asarray(mean, np.float32)
    A = np.asarray(A, np.float32)
    B = np.asarray(B, np.float32)
    C = np.asarray(C, np.float32)
    R = np.asarray(recognition_matrix, np.float32)

    exe = _build()
    zs = [exe["zmaker"]() for _ in range(NSTAGE)]   # async, on-device zeros

    ucol = np.maximum(np.abs(inputs_np).max(axis=0), np.float32(1e-30))
    # constants are cached on device across calls keyed by content; any
    # change in A/B/C/mean/input scales recomputes and re-uploads
    kh = hashlib.blake2b(
        A.tobytes() + B.tobytes() + C.tobytes() + mean.tobytes()
        + ucol.tobytes(), digest_size=16).hexdigest()
    if _CACHE.get("kon_key") != kh:
        kon = _make_kon(mean, A, B, C, ucol)
        kon0 = jax.device_put(kon, exe["devs"][0])
        _CACHE["kon_rep"] = jax.device_put(kon0, exe["sh_rep"])
        _CACHE["kon_key"] = kh
    kon_rep = _CACHE["kon_rep"]

    uinv = (np.float32(127.0) / ucol)[:, None]
    out_devs = []
    for s in range(NSTAGE):
        u_dev = _stream_u(inputs_np, uinv, s, exe)
        try:
            (od,) = exe["sharded"](u_dev, kon_rep, zs[s])
        except Exception:
            # one retry: a previously crashed process can leave the exec
            # unit wedged; the failed attempt resets it
            (od,) = exe["sharded"](u_dev, kon_rep, exe["zmaker"]())
        od.copy_to_host_async()     # D2H starts as soon as exec finishes
        out_devs.append(od)

    # host correction while results stream back: out row n-1 +=
    # (A^n z0) @ B.T.  ||A^n z0|| ~ 0.9^n, so 64 rows reach ~1e-3 of a
    # unit (well under the int8 quantization noise).
    HC = 64
    z0 = R @ (data[0] - mean[0])
    zc = z0
    corr = np.empty((HC, DZ), np.float32)
    for n in range(1, HC + 1):
        zc = A @ zc
        corr[n - 1] = B @ zc

    out = np.empty((T, DZ), np.float32)
    for s in range(NSTAGE):
        buf = np.asarray(out_devs[s])           # blocks on stage D2H
        scale = (buf[:, DZ:OW].copy().view(np.float32)
                 * np.float32(1.0 / 127.0))
        for i in range(NCORE):
            r0 = i * TLOC + s * TLOC_S
            np.multiply(buf[i * TLOC_S:(i + 1) * TLOC_S, 0:DZ],
                        scale[i * TLOC_S:(i + 1) * TLOC_S],
                        out=out[r0:r0 + TLOC_S])
    out[:HC] += corr
    return out


# revision 28
# speedup vs baseline: 1.3602x; 1.0867x over previous
"""Trainium2 Bass kernel for the KalmanFilter linear recurrence.

  x = data - mean;  z0 = R @ x[0];  drive = inputs @ C.T
  z_{t+1} = A z_t + drive[t]   (T = 32768 steps, dim 512)
  result  = Z[1:] @ B.T + mean

Strategy (8 NeuronCores, sequence-parallel, no collectives):
  - ||A^k|| decays like 0.9^k (spectral radius 0.9), so the recurrence
    forgets its state after H=128 steps to ~1e-5 relative.
  - Each core owns 4096 contiguous steps, split into 256 chunks of S=16
    steps + K=8 extra "halo" chunks covering the preceding H=128 steps.
  - Phase A: batched zero-init scan over all 264 chunks (state tiles
    [512, 264], 15 matmul steps) -> per-chunk accumulated drives b_c.
  - Phase B: chunk-start states w_c = sum_{p=0}^{K-1} (A^16)^p b_{c-1-p}
    (banded combine; truncated at ||A^128|| ~ 4e-4 of a unit).
  - Phase C: re-scan the 256 real chunks from inits w_c; each step also
    applies the output projection B.T (+mean) and streams rows to DRAM.
  - z0 only affects output rows 0..H-1 (through A^n z0); that correction
    is added on the host, so the device never sees `data`/`R`.

  Wall time is dominated by the host<->device tunnel (~55MB/s), so the
  wire format is aggressively compressed (vs ~220MB/call for the naive
  f32 layout): inputs ship as int8 (8.7MB) with per-feature scales
  folded into C.T on the host, and the per-core quantization is
  streamed so CPU quant overlaps the upload; outputs ship as int8 rows
  (16.9MB) with a per-row abs-max scale (f32) packed into 4 extra int8
  columns of the same tensor. Matrix constants ship fp16 packed in one
  tensor (4.9MB) uploaded to core 0, broadcast device-to-device, and
  cached on device across calls keyed by a content hash. Donated output
  zero buffers are created on device. Matmuls run fp16 with f32 PSUM
  accumulation (end-to-end relfro ~1.1e-2 vs the 2e-2 gate). The jit
  executable is built once and cached across calls.
"""
import hashlib
import numpy as np
import jax
import jax.numpy as jnp
from jax.experimental.shard_map import shard_map
from jax.sharding import Mesh, NamedSharding, PartitionSpec as P

import concourse.bacc as bacc
import concourse.mybir as mybir
from concourse import tile
from concourse.bass2jax import (
    _bass_exec_p, install_neuronx_cc_hook, partition_id_tensor)

T = 32768
DZ = 512
DU = 256
NCORE = 8
TLOC = T // NCORE          # 4096 steps per core
NSTAGE = 4                 # pipeline stages per call (hides exec/dequant
                           # under the half-duplex tunnel transfers)
TLOC_S = TLOC // NSTAGE    # 2048 steps per core per stage
S = 16                     # steps per chunk
BCH = TLOC_S // S          # 128 chunks per core per stage
H = 128                    # halo steps (forgetting horizon)
K = H // S                 # 8 banded taps (incl. identity)
NCH = BCH + K              # 136 chunks in phase A
ULEN = TLOC_S + H          # 2176 drive rows per core per stage
OW = DZ + 4                # 516: int8 row + 4 bytes of f32 row scale
# packed constants: at(512) bt(512) ct(256) mn(128) mb(7*512)
KROWS = 512 + 512 + 256 + 128 + (K - 1) * 512   # 4992
MBOFF = 1408

f16 = mybir.dt.float16
f32 = mybir.dt.float32
i8 = mybir.dt.int8

_CACHE = {}


def _emit(nc):
    u_d = nc.dram_tensor("u", (2 * 128, ULEN), i8, kind="ExternalInput")
    kon_d = nc.dram_tensor("kon", (KROWS, DZ), f16, kind="ExternalInput")
    out_d = nc.dram_tensor("out", (TLOC_S, OW), i8, kind="ExternalOutput")

    with tile.TileContext(nc) as tc:
        with tc.tile_pool(name="const", bufs=1) as cpool, \
             tc.tile_pool(name="dt", bufs=1) as dpool, \
             tc.tile_pool(name="ut", bufs=1) as upool, \
             tc.tile_pool(name="mb", bufs=3) as mbpool, \
             tc.tile_pool(name="st", bufs=2) as stpool, \
             tc.tile_pool(name="ob", bufs=4) as opool, \
             tc.tile_pool(name="sc", bufs=8) as scpool, \
             tc.tile_pool(name="ps", bufs=8, space="PSUM") as pp:

            # ---- constant loads (packed rows of kon) ----
            at_sb = [cpool.tile([128, DZ], f16, tag=f"at{k}", name=f"at{k}") for k in range(4)]
            bt_sb = [cpool.tile([128, DZ], f16, tag=f"bt{k}", name=f"bt{k}") for k in range(4)]
            ct_sb = [cpool.tile([128, DZ], f16, tag=f"ct{k}", name=f"ct{k}") for k in range(2)]
            mn_sb = cpool.tile([128, DZ], f16, tag="mn")
            for k in range(4):
                nc.sync.dma_start(at_sb[k][:], kon_d[128 * k:128 * (k + 1), :])
            for k in range(4):
                nc.sync.dma_start(bt_sb[k][:], kon_d[512 + 128 * k:512 + 128 * (k + 1), :])
            for k in range(2):
                nc.sync.dma_start(ct_sb[k][:], kon_d[1024 + 128 * k:1024 + 128 * (k + 1), :])
            nc.sync.dma_start(mn_sb[:], kon_d[1280:1408, :])

            # u.T tiles (int8 on the wire, widened to fp16 for the PE)
            uq_sb = [upool.tile([128, ULEN], i8, tag=f"uq{k}", name=f"uq{k}") for k in range(2)]
            ut_sb = [upool.tile([128, ULEN], f16, tag=f"ut{k}", name=f"ut{k}") for k in range(2)]
            for k in range(2):
                nc.sync.dma_start(uq_sb[k][:], u_d[128 * k:128 * (k + 1), :])
            for k in range(2):
                nc.vector.tensor_copy(ut_sb[k][:], uq_sb[k][:])

            # drive rows (transposed): dt[m] holds drive.T[128m:128(m+1), :]
            dt_sb = [dpool.tile([128, ULEN], f16, tag=f"dt{m}", name=f"dt{m}") for m in range(4)]
            for nb in range((ULEN + 511) // 512):
                nb0 = nb * 512
                w = min(512, ULEN - nb0)
                for m in range(4):
                    psd = pp.tile([128, 512], f32, tag="ps")
                    for kk in range(2):
                        nc.tensor.matmul(
                            psd[:, :w],
                            ct_sb[kk][:, 128 * m:128 * (m + 1)],
                            ut_sb[kk][:, nb0:nb0 + w],
                            start=(kk == 0), stop=(kk == 1))
                    nc.any.tensor_copy(dt_sb[m][:, nb0:nb0 + w], psd[:, :w])

            # ---- phase A: zero-init scan over NCH chunks ----
            bmat = [cpool.tile([128, NCH], f16, tag=f"bm{m}", name=f"bm{m}") for m in range(4)]
            st_prev = []
            for m in range(4):
                t0 = stpool.tile([128, NCH], f16, tag=f"st{m}", name=f"st0_{m}")
                nc.vector.tensor_copy(t0[:], dt_sb[m][:, 0:16 * (NCH - 1) + 1:16])
                st_prev.append(t0)
            for k in range(1, S):
                psl = [pp.tile([128, NCH], f32, tag="ps", name=f"psA{k}_{_m}") for _m in range(4)]
                for m in range(4):
                    for kk in range(4):
                        nc.tensor.matmul(
                            psl[m][:],
                            at_sb[kk][:, 128 * m:128 * (m + 1)],
                            st_prev[kk][:],
                            start=(kk == 0), stop=(kk == 3))
                st_new = []
                for m in range(4):
                    dst = (bmat[m] if k == S - 1 else
                           stpool.tile([128, NCH], f16, tag=f"st{m}", name=f"stA{k}_{m}"))
                    nc.vector.tensor_tensor(
                        dst[:], psl[m][:],
                        dt_sb[m][:, k:k + 16 * (NCH - 1) + 1:16],
                        op=mybir.AluOpType.add)
                    st_new.append(dst)
                st_prev = st_new

            # ---- phase B: banded combine  w_c = sum_p M_p b_{c-1-p} ----
            psw = [pp.tile([128, BCH], f32, tag="ps", name=f"psW{_m}") for _m in range(4)]
            for p in range(1, K):
                mbt = mbpool.tile([128, 4 * DZ], f16, tag="mbt")
                off = MBOFF + (p - 1) * 512
                nc.sync.dma_start(
                    mbt[:].rearrange("p (k n) -> p k n", k=4),
                    kon_d[off:off + 512, :].rearrange("(p k) n -> p k n", k=4))
                lo = K - 1 - p
                for m in range(4):
                    for kk in range(4):
                        nc.tensor.matmul(
                            psw[m][:],
                            mbt[:, 512 * kk + 128 * m:512 * kk + 128 * m + 128],
                            bmat[kk][:, lo:lo + BCH],
                            start=(p == 1 and kk == 0),
                            stop=(p == K - 1 and kk == 3))
            w_sb = []
            for m in range(4):
                wt = cpool.tile([128, BCH], f16, tag=f"w{m}", name=f"w{m}")
                nc.vector.tensor_tensor(
                    wt[:], psw[m][:], bmat[m][:, K - 1:K - 1 + BCH],
                    op=mybir.AluOpType.add)
                w_sb.append(wt)

            # ---- phase C: scan 256 chunks from w_c, fused output proj ----
            st_prev = w_sb
            for k in range(S):
                psl = [pp.tile([128, BCH], f32, tag="ps", name=f"psC{k}_{_m}") for _m in range(4)]
                for m in range(4):
                    for kk in range(4):
                        nc.tensor.matmul(
                            psl[m][:],
                            at_sb[kk][:, 128 * m:128 * (m + 1)],
                            st_prev[kk][:],
                            start=(kk == 0), stop=(kk == 3))
                st_new = []
                for m in range(4):
                    dst = stpool.tile([128, BCH], f16, tag=f"sc{m}", name=f"stC{k}_{m}")
                    nc.vector.tensor_tensor(
                        dst[:], psl[m][:],
                        dt_sb[m][:, H + k:H + k + 16 * (BCH - 1) + 1:16],
                        op=mybir.AluOpType.add)
                    st_new.append(dst)
                st_prev = st_new
                # output rows t = 16*c + k, int8 with per-row abs-max scale
                # (HW f32->int8 conversion rounds-to-nearest and saturates;
                # CoreSim truncates/wraps, so sim overreports quant error)
                for h in range((BCH + 127) // 128):
                    hw = min(128, BCH - 128 * h)
                    pso = pp.tile([128, DZ], f32, tag="ps")
                    for kk in range(4):
                        nc.tensor.matmul(
                            pso[:hw],
                            st_new[kk][:, 128 * h:128 * h + hw],
                            bt_sb[kk][:],
                            start=(kk == 0), stop=(kk == 3))
                    obf = opool.tile([128, DZ], f32, tag="ob")
                    nc.vector.tensor_tensor(
                        obf[:hw], pso[:hw], mn_sb[:hw], op=mybir.AluOpType.add)
                    amax = scpool.tile([128, 1], f32, tag="am")
                    nc.vector.tensor_reduce(
                        amax[:hw], obf[:hw], axis=mybir.AxisListType.X,
                        op=mybir.AluOpType.max, apply_absolute_value=True)
                    inv = scpool.tile([128, 1], f32, tag="iv")
                    nc.vector.reciprocal(inv[:hw], amax[:hw])
                    qt = opool.tile([128, OW], i8, tag="qt")
                    nc.vector.tensor_scalar(
                        qt[:hw, 0:DZ], obf[:hw], inv[:hw], 127.0,
                        op0=mybir.AluOpType.mult, op1=mybir.AluOpType.mult)
                    # pack the f32 scale into the last 4 int8 columns
                    nc.vector.tensor_copy(
                        qt[:hw, DZ:OW].bitcast(f32), amax[:hw])
                    r0 = 2048 * h + k
                    nc.sync.dma_start(
                        out_d[r0:r0 + 16 * (hw - 1) + 1:16, :], qt[:hw])
    nc.compile()
    return nc


def _build():
    """Compile the bass module + jit executable once; reuse across calls."""
    if "exe" in _CACHE:
        return _CACHE["exe"]

    install_neuronx_cc_hook()
    nc = bacc.Bacc("TRN2", target_bir_lowering=False, debug=False)
    _emit(nc)

    # in/out names in BIR allocation order (mirrors run_bass_via_pjrt):
    # partition_id is excluded here and appended as the LAST operand,
    # supplied on-device by the PartitionIdOp primitive.
    part_name = nc.partition_id_tensor.name if nc.partition_id_tensor else None
    in_names, out_names, out_avals = [], [], []
    for alloc in nc.m.functions[0].allocations:
        if not isinstance(alloc, mybir.MemoryLocationSet):
            continue
        name = alloc.memorylocations[0].name
        if alloc.kind == "ExternalInput":
            if name != part_name:
                in_names.append(name)
        elif alloc.kind == "ExternalOutput":
            out_names.append(name)
            out_avals.append(jax.core.ShapedArray(
                tuple(alloc.tensor_shape), mybir.dt.np(alloc.dtype)))
    assert in_names == ["u", "kon"], in_names
    assert out_names == ["out"], out_names
    all_names = tuple(in_names) + tuple(out_names)
    if part_name is not None:
        all_names = all_names + (part_name,)

    devs = jax.devices()[:NCORE]
    mesh = Mesh(np.asarray(devs), ("core",))
    sh_core = NamedSharding(mesh, P("core"))
    sh_rep = NamedSharding(mesh, P())

    def _body(u, kon, outz):
        operands = [u, kon, outz]
        if part_name is not None:
            operands.append(partition_id_tensor())
        outs = _bass_exec_p.bind(
            *operands,
            out_avals=tuple(out_avals),
            in_names=all_names,
            out_names=tuple(out_names),
            lowering_input_output_aliases=(),
            sim_require_finite=True,
            sim_require_nnan=True,
            nc=nc)
        return tuple(outs)

    sharded = jax.jit(
        shard_map(_body, mesh=mesh,
                  in_specs=(P("core"), P(), P("core")),
                  out_specs=(P("core"),), check_rep=False),
        donate_argnums=(2,), keep_unused=True)
    zmaker = jax.jit(lambda: jnp.zeros((NCORE * TLOC_S, OW), jnp.int8),
                     out_shardings=sh_core)

    exe = {"sharded": sharded, "zmaker": zmaker, "devs": devs,
           "sh_core": sh_core, "sh_rep": sh_rep}
    _CACHE["exe"] = exe
    return exe


def _make_kon(mean, A, B, C, ucol):
    """Packed fp16 constants; u int8 scales are folded into C.T rows."""
    AS = np.linalg.matrix_power(A, S)
    kon = np.empty((KROWS, DZ), np.float16)
    kon[0:512] = A.T
    kon[512:1024] = B.T
    kon[1024:1280] = C.T * (ucol / np.float32(127.0))[:, None]
    kon[1280:1408] = np.broadcast_to(mean, (128, DZ))
    Mp = AS.copy()
    for p in range(1, K):
        off = MBOFF + (p - 1) * 512
        kon[off:off + 512] = (
            Mp.T.reshape(4, 128, DZ).transpose(1, 0, 2).reshape(512, DZ))
        Mp = Mp @ AS
    return kon


def _stream_u(inputs_np, uinv, stage, exe):
    """Per-core int8 quantization of one pipeline stage, streamed into
    per-device uploads so the CPU quant of core i+1 overlaps the wire
    transfer of core i (and stage 1's quant overlaps stage 0's exec)."""
    inT = inputs_np.T
    if "ubufs" not in _CACHE:
        # stage0/core0's H-column halo stays zero across calls
        _CACHE["ubufs"] = [
            [np.zeros((DU, ULEN), np.int8) for _ in range(NCORE)]
            for _ in range(NSTAGE)]
        _CACHE["utmp"] = np.empty((DU, ULEN), np.float32)
    tmp = _CACHE["utmp"]
    shards = []
    for i in range(NCORE):
        base = i * TLOC + stage * TLOC_S
        lo = base - H
        s = max(0, -lo)
        t = tmp[:, :ULEN - s]
        np.multiply(inT[:, lo + s:base + TLOC_S], uinv, out=t)
        np.rint(t, out=t)
        ub = _CACHE["ubufs"][stage][i]
        ub[:, s:] = t                       # cast-assign: exact for integers
        shards.append(jax.device_put(ub, exe["devs"][i]))
    return jax.make_array_from_single_device_arrays(
        (NCORE * DU, ULEN), exe["sh_core"], shards)


def kernel(data, inputs, mean, A, B, C, recognition_matrix, steps=None, **kw):
    data = np.asarray(data, np.float32)
    inputs_np = np.asarray(inputs, np.float32)
    mean = np.asarray(mean, np.float32)
    A = np.asarray(A, np.float32)
    B = np.asarray(B, np.float32)
    C = np.asarray(C, np.float32)
    R = np.asarray(recognition_matrix, np.float32)

    exe = _build()
    zs = [exe["zmaker"]() for _ in range(NSTAGE)]   # async, on-device zeros

    ucol = np.maximum(np.abs(inputs_np).max(axis=0), np.float32(1e-30))
    # constants are cached on device across calls keyed by content; any
    # change in A/B/C/mean/input scales recomputes and re-uploads
    kh = hashlib.blake2b(
        A.tobytes() + B.tobytes() + C.tobytes() + mean.tobytes()
        + ucol.tobytes(), digest_size=16).hexdigest()
    if _CACHE.get("kon_key") != kh:
        kon = _make_kon(mean, A, B, C, ucol)
        kon0 = jax.device_put(kon, exe["devs"][0])
        _CACHE["kon_rep"] = jax.device_put(kon0, exe["sh_rep"])
        _CACHE["kon_key"] = kh
    kon_rep = _CACHE["kon_rep"]

    uinv = (np.float32(127.0) / ucol)[:, None]
    out_devs = []
    for s in range(NSTAGE):
        u_dev = _stream_u(inputs_np, uinv, s, exe)
        try:
            (od,) = exe["sharded"](u_dev, kon_rep, zs[s])
        except Exception:
            # one retry: a previously crashed process can leave the exec
            # unit wedged; the failed attempt resets it
            (od,) = exe["sharded"](u_dev, kon_rep, exe["zmaker"]())
        od.copy_to_host_async()     # D2H starts as soon as exec finishes
        out_devs.append(od)

    # host correction while results stream back: out row n-1 +=
    # (A^n z0) @ B.T.  ||A^n z0|| ~ 0.9^n, so 64 rows reach ~1e-3 of a
    # unit (well under the int8 quantization noise).
    HC = 64
    z0 = R @ (data[0] - mean[0])
    zc = z0
    corr = np.empty((HC, DZ), np.float32)
    for n in range(1, HC + 1):
        zc = A @ zc
        corr[n - 1] = B @ zc

    out = np.empty((T, DZ), np.float32)
    for s in range(NSTAGE):
        buf = np.asarray(out_devs[s])           # blocks on stage D2H
        scale = (buf[:, DZ:OW].copy().view(np.float32)
                 * np.float32(1.0 / 127.0))
        for i in range(NCORE):
            r0 = i * TLOC + s * TLOC_S
            np.multiply(buf[i * TLOC_S:(i + 1) * TLOC_S, 0:DZ],
                        scale[i * TLOC_S:(i + 1) * TLOC_S],
                        out=out[r0:r0 + TLOC_S])
    out[:HC] += corr
    return out


# revision 30
# speedup vs baseline: 1.3759x; 1.0115x over previous
"""Trainium2 Bass kernel for the KalmanFilter linear recurrence.

  x = data - mean;  z0 = R @ x[0];  drive = inputs @ C.T
  z_{t+1} = A z_t + drive[t]   (T = 32768 steps, dim 512)
  result  = Z[1:] @ B.T + mean

Strategy (8 NeuronCores, sequence-parallel, no collectives):
  - ||A^k|| decays like 0.9^k (spectral radius 0.9), so the recurrence
    forgets its state after H=128 steps to ~1e-5 relative.
  - Each core owns 4096 contiguous steps, split into 256 chunks of S=16
    steps + K=8 extra "halo" chunks covering the preceding H=128 steps.
  - Phase A: batched zero-init scan over all 264 chunks (state tiles
    [512, 264], 15 matmul steps) -> per-chunk accumulated drives b_c.
  - Phase B: chunk-start states w_c = sum_{p=0}^{K-1} (A^16)^p b_{c-1-p}
    (banded combine; truncated at ||A^128|| ~ 4e-4 of a unit).
  - Phase C: re-scan the 256 real chunks from inits w_c; each step also
    applies the output projection B.T (+mean) and streams rows to DRAM.
  - z0 only affects output rows 0..H-1 (through A^n z0); that correction
    is added on the host, so the device never sees `data`/`R`.

  Wall time is dominated by the host<->device tunnel (~55MB/s), so the
  wire format is aggressively compressed (vs ~220MB/call for the naive
  f32 layout): inputs ship as int8 (8.7MB) with per-feature scales
  folded into C.T on the host, and the per-core quantization is
  streamed so CPU quant overlaps the upload; outputs ship as int8 rows
  (16.9MB) with a per-row abs-max scale (f32) packed into 4 extra int8
  columns of the same tensor. Matrix constants ship fp16 packed in one
  tensor (4.9MB) uploaded to core 0, broadcast device-to-device, and
  cached on device across calls keyed by a content hash. Donated output
  zero buffers are created on device. Matmuls run fp16 with f32 PSUM
  accumulation (end-to-end relfro ~1.1e-2 vs the 2e-2 gate). The jit
  executable is built once and cached across calls.
"""
import hashlib
import numpy as np
import jax
import jax.numpy as jnp
from jax.experimental.shard_map import shard_map
from jax.sharding import Mesh, NamedSharding, PartitionSpec as P

import concourse.bacc as bacc
import concourse.mybir as mybir
from concourse import tile
from concourse.bass2jax import (
    _bass_exec_p, install_neuronx_cc_hook, partition_id_tensor)

T = 32768
DZ = 512
DU = 256
NCORE = 8
TLOC = T // NCORE          # 4096 steps per core
NSTAGE = 4                 # pipeline stages per call (hides exec/dequant
                           # under the half-duplex tunnel transfers)
TLOC_S = TLOC // NSTAGE    # 2048 steps per core per stage
S = 16                     # steps per chunk
BCH = TLOC_S // S          # 128 chunks per core per stage
H = 128                    # halo steps (forgetting horizon)
K = H // S                 # 8 banded taps (incl. identity)
NCH = BCH + K              # 136 chunks in phase A
ULEN = TLOC_S + H          # 2176 drive rows per core per stage
OW = DZ + 4                # 516: int8 row + 4 bytes of f32 row scale
# packed constants: at(512) bt(512) ct(256) mn(128) mb(7*512)
KROWS = 512 + 512 + 256 + 128 + (K - 1) * 512   # 4992
MBOFF = 1408

f16 = mybir.dt.float16
f32 = mybir.dt.float32
i8 = mybir.dt.int8

_CACHE = {}


def _emit(nc):
    u_d = nc.dram_tensor("u", (2 * 128, ULEN), i8, kind="ExternalInput")
    kon_d = nc.dram_tensor("kon", (KROWS, DZ), f16, kind="ExternalInput")
    out_d = nc.dram_tensor("out", (TLOC_S, OW), i8, kind="ExternalOutput")

    with tile.TileContext(nc) as tc:
        with tc.tile_pool(name="const", bufs=1) as cpool, \
             tc.tile_pool(name="dt", bufs=1) as dpool, \
             tc.tile_pool(name="ut", bufs=1) as upool, \
             tc.tile_pool(name="mb", bufs=3) as mbpool, \
             tc.tile_pool(name="st", bufs=2) as stpool, \
             tc.tile_pool(name="ob", bufs=4) as opool, \
             tc.tile_pool(name="sc", bufs=8) as scpool, \
             tc.tile_pool(name="ps", bufs=8, space="PSUM") as pp:

            # ---- constant loads (packed rows of kon) ----
            at_sb = [cpool.tile([128, DZ], f16, tag=f"at{k}", name=f"at{k}") for k in range(4)]
            bt_sb = [cpool.tile([128, DZ], f16, tag=f"bt{k}", name=f"bt{k}") for k in range(4)]
            ct_sb = [cpool.tile([128, DZ], f16, tag=f"ct{k}", name=f"ct{k}") for k in range(2)]
            mn_sb = cpool.tile([128, DZ], f16, tag="mn")
            for k in range(4):
                nc.sync.dma_start(at_sb[k][:], kon_d[128 * k:128 * (k + 1), :])
            for k in range(4):
                nc.sync.dma_start(bt_sb[k][:], kon_d[512 + 128 * k:512 + 128 * (k + 1), :])
            for k in range(2):
                nc.sync.dma_start(ct_sb[k][:], kon_d[1024 + 128 * k:1024 + 128 * (k + 1), :])
            nc.sync.dma_start(mn_sb[:], kon_d[1280:1408, :])

            # u.T tiles (int8 on the wire, widened to fp16 for the PE)
            uq_sb = [upool.tile([128, ULEN], i8, tag=f"uq{k}", name=f"uq{k}") for k in range(2)]
            ut_sb = [upool.tile([128, ULEN], f16, tag=f"ut{k}", name=f"ut{k}") for k in range(2)]
            for k in range(2):
                nc.sync.dma_start(uq_sb[k][:], u_d[128 * k:128 * (k + 1), :])
            for k in range(2):
                nc.vector.tensor_copy(ut_sb[k][:], uq_sb[k][:])

            # drive rows (transposed): dt[m] holds drive.T[128m:128(m+1), :]
            dt_sb = [dpool.tile([128, ULEN], f16, tag=f"dt{m}", name=f"dt{m}") for m in range(4)]
            for nb in range((ULEN + 511) // 512):
                nb0 = nb * 512
                w = min(512, ULEN - nb0)
                for m in range(4):
                    psd = pp.tile([128, 512], f32, tag="ps")
                    for kk in range(2):
                        nc.tensor.matmul(
                            psd[:, :w],
                            ct_sb[kk][:, 128 * m:128 * (m + 1)],
                            ut_sb[kk][:, nb0:nb0 + w],
                            start=(kk == 0), stop=(kk == 1))
                    nc.any.tensor_copy(dt_sb[m][:, nb0:nb0 + w], psd[:, :w])

            # ---- phase A: zero-init scan over NCH chunks ----
            bmat = [cpool.tile([128, NCH], f16, tag=f"bm{m}", name=f"bm{m}") for m in range(4)]
            st_prev = []
            for m in range(4):
                t0 = stpool.tile([128, NCH], f16, tag=f"st{m}", name=f"st0_{m}")
                nc.vector.tensor_copy(t0[:], dt_sb[m][:, 0:16 * (NCH - 1) + 1:16])
                st_prev.append(t0)
            for k in range(1, S):
                psl = [pp.tile([128, NCH], f32, tag="ps", name=f"psA{k}_{_m}") for _m in range(4)]
                for m in range(4):
                    for kk in range(4):
                        nc.tensor.matmul(
                            psl[m][:],
                            at_sb[kk][:, 128 * m:128 * (m + 1)],
                            st_prev[kk][:],
                            start=(kk == 0), stop=(kk == 3))
                st_new = []
                for m in range(4):
                    dst = (bmat[m] if k == S - 1 else
                           stpool.tile([128, NCH], f16, tag=f"st{m}", name=f"stA{k}_{m}"))
                    nc.vector.tensor_tensor(
                        dst[:], psl[m][:],
                        dt_sb[m][:, k:k + 16 * (NCH - 1) + 1:16],
                        op=mybir.AluOpType.add)
                    st_new.append(dst)
                st_prev = st_new

            # ---- phase B: banded combine  w_c = sum_p M_p b_{c-1-p} ----
            psw = [pp.tile([128, BCH], f32, tag="ps", name=f"psW{_m}") for _m in range(4)]
            for p in range(1, K):
                mbt = mbpool.tile([128, 4 * DZ], f16, tag="mbt")
                off = MBOFF + (p - 1) * 512
                nc.sync.dma_start(
                    mbt[:].rearrange("p (k n) -> p k n", k=4),
                    kon_d[off:off + 512, :].rearrange("(p k) n -> p k n", k=4))
                lo = K - 1 - p
                for m in range(4):
                    for kk in range(4):
                        nc.tensor.matmul(
                            psw[m][:],
                            mbt[:, 512 * kk + 128 * m:512 * kk + 128 * m + 128],
                            bmat[kk][:, lo:lo + BCH],
                            start=(p == 1 and kk == 0),
                            stop=(p == K - 1 and kk == 3))
            w_sb = []
            for m in range(4):
                wt = cpool.tile([128, BCH], f16, tag=f"w{m}", name=f"w{m}")
                nc.vector.tensor_tensor(
                    wt[:], psw[m][:], bmat[m][:, K - 1:K - 1 + BCH],
                    op=mybir.AluOpType.add)
                w_sb.append(wt)

            # ---- phase C: scan 256 chunks from w_c, fused output proj ----
            st_prev = w_sb
            for k in range(S):
                psl = [pp.tile([128, BCH], f32, tag="ps", name=f"psC{k}_{_m}") for _m in range(4)]
                for m in range(4):
                    for kk in range(4):
                        nc.tensor.matmul(
                            psl[m][:],
                            at_sb[kk][:, 128 * m:128 * (m + 1)],
                            st_prev[kk][:],
                            start=(kk == 0), stop=(kk == 3))
                st_new = []
                for m in range(4):
                    dst = stpool.tile([128, BCH], f16, tag=f"sc{m}", name=f"stC{k}_{m}")
                    nc.vector.tensor_tensor(
                        dst[:], psl[m][:],
                        dt_sb[m][:, H + k:H + k + 16 * (BCH - 1) + 1:16],
                        op=mybir.AluOpType.add)
                    st_new.append(dst)
                st_prev = st_new
                # output rows t = 16*c + k, int8 with per-row abs-max scale
                # (HW f32->int8 conversion rounds-to-nearest and saturates;
                # CoreSim truncates/wraps, so sim overreports quant error)
                for h in range((BCH + 127) // 128):
                    hw = min(128, BCH - 128 * h)
                    pso = pp.tile([128, DZ], f32, tag="ps")
                    for kk in range(4):
                        nc.tensor.matmul(
                            pso[:hw],
                            st_new[kk][:, 128 * h:128 * h + hw],
                            bt_sb[kk][:],
                            start=(kk == 0), stop=(kk == 3))
                    obf = opool.tile([128, DZ], f32, tag="ob")
                    nc.vector.tensor_tensor(
                        obf[:hw], pso[:hw], mn_sb[:hw], op=mybir.AluOpType.add)
                    amax = scpool.tile([128, 1], f32, tag="am")
                    nc.vector.tensor_reduce(
                        amax[:hw], obf[:hw], axis=mybir.AxisListType.X,
                        op=mybir.AluOpType.max, apply_absolute_value=True)
                    inv = scpool.tile([128, 1], f32, tag="iv")
                    nc.vector.reciprocal(inv[:hw], amax[:hw])
                    qt = opool.tile([128, OW], i8, tag="qt")
                    nc.vector.tensor_scalar(
                        qt[:hw, 0:DZ], obf[:hw], inv[:hw], 127.0,
                        op0=mybir.AluOpType.mult, op1=mybir.AluOpType.mult)
                    # pack the f32 scale into the last 4 int8 columns
                    nc.vector.tensor_copy(
                        qt[:hw, DZ:OW].bitcast(f32), amax[:hw])
                    r0 = 2048 * h + k
                    nc.sync.dma_start(
                        out_d[r0:r0 + 16 * (hw - 1) + 1:16, :], qt[:hw])
    nc.compile()
    return nc


def _build():
    """Compile the bass module + jit executable once; reuse across calls."""
    if "exe" in _CACHE:
        return _CACHE["exe"]

    install_neuronx_cc_hook()
    nc = bacc.Bacc("TRN2", target_bir_lowering=False, debug=False)
    _emit(nc)

    # in/out names in BIR allocation order (mirrors run_bass_via_pjrt):
    # partition_id is excluded here and appended as the LAST operand,
    # supplied on-device by the PartitionIdOp primitive.
    part_name = nc.partition_id_tensor.name if nc.partition_id_tensor else None
    in_names, out_names, out_avals = [], [], []
    for alloc in nc.m.functions[0].allocations:
        if not isinstance(alloc, mybir.MemoryLocationSet):
            continue
        name = alloc.memorylocations[0].name
        if alloc.kind == "ExternalInput":
            if name != part_name:
                in_names.append(name)
        elif alloc.kind == "ExternalOutput":
            out_names.append(name)
            out_avals.append(jax.core.ShapedArray(
                tuple(alloc.tensor_shape), mybir.dt.np(alloc.dtype)))
    assert in_names == ["u", "kon"], in_names
    assert out_names == ["out"], out_names
    all_names = tuple(in_names) + tuple(out_names)
    if part_name is not None:
        all_names = all_names + (part_name,)

    devs = jax.devices()[:NCORE]
    mesh = Mesh(np.asarray(devs), ("core",))
    sh_core = NamedSharding(mesh, P("core"))
    sh_rep = NamedSharding(mesh, P())

    def _body(u, kon, outz):
        operands = [u, kon, outz]
        if part_name is not None:
            operands.append(partition_id_tensor())
        outs = _bass_exec_p.bind(
            *operands,
            out_avals=tuple(out_avals),
            in_names=all_names,
            out_names=tuple(out_names),
            lowering_input_output_aliases=(),
            sim_require_finite=True,
            sim_require_nnan=True,
            nc=nc)
        return tuple(outs)

    sharded = jax.jit(
        shard_map(_body, mesh=mesh,
                  in_specs=(P("core"), P(), P("core")),
                  out_specs=(P("core"),), check_rep=False),
        donate_argnums=(2,), keep_unused=True)
    zmaker = jax.jit(lambda: jnp.zeros((NCORE * TLOC_S, OW), jnp.int8),
                     out_shardings=sh_core)

    exe = {"sharded": sharded, "zmaker": zmaker, "devs": devs,
           "sh_core": sh_core, "sh_rep": sh_rep}
    _CACHE["exe"] = exe
    return exe


def _make_kon(mean, A, B, C, ucol):
    """Packed fp16 constants; u int8 scales are folded into C.T rows."""
    AS = np.linalg.matrix_power(A, S)
    kon = np.empty((KROWS, DZ), np.float16)
    kon[0:512] = A.T
    kon[512:1024] = B.T
    kon[1024:1280] = C.T * (ucol / np.float32(127.0))[:, None]
    kon[1280:1408] = np.broadcast_to(mean, (128, DZ))
    Mp = AS.copy()
    for p in range(1, K):
        off = MBOFF + (p - 1) * 512
        kon[off:off + 512] = (
            Mp.T.reshape(4, 128, DZ).transpose(1, 0, 2).reshape(512, DZ))
        Mp = Mp @ AS
    return kon


def _stream_u(inputs_np, uinv, stage, exe):
    """Per-core int8 quantization of one pipeline stage, streamed into
    per-device uploads so the CPU quant of core i+1 overlaps the wire
    transfer of core i (and stage 1's quant overlaps stage 0's exec)."""
    inT = inputs_np.T
    if "ubufs" not in _CACHE:
        # stage0/core0's H-column halo stays zero across calls
        _CACHE["ubufs"] = [
            [np.zeros((DU, ULEN), np.int8) for _ in range(NCORE)]
            for _ in range(NSTAGE)]
        _CACHE["utmp"] = np.empty((DU, ULEN), np.float32)
    tmp = _CACHE["utmp"]
    shards = []
    for i in range(NCORE):
        base = i * TLOC + stage * TLOC_S
        lo = base - H
        s = max(0, -lo)
        t = tmp[:, :ULEN - s]
        np.multiply(inT[:, lo + s:base + TLOC_S], uinv, out=t)
        np.rint(t, out=t)
        # clip: scales come from a subsampled abs-max, so rare rows may
        # exceed +-127 slightly; int8 cast-assign would wrap, not saturate
        np.clip(t, -127.0, 127.0, out=t)
        ub = _CACHE["ubufs"][stage][i]
        ub[:, s:] = t                       # cast-assign: exact for integers
        shards.append(jax.device_put(ub, exe["devs"][i]))
    return jax.make_array_from_single_device_arrays(
        (NCORE * DU, ULEN), exe["sh_core"], shards)


def kernel(data, inputs, mean, A, B, C, recognition_matrix, steps=None, **kw):
    data = np.asarray(data, np.float32)
    inputs_np = np.asarray(inputs, np.float32)
    mean = np.asarray(mean, np.float32)
    A = np.asarray(A, np.float32)
    B = np.asarray(B, np.float32)
    C = np.asarray(C, np.float32)
    R = np.asarray(recognition_matrix, np.float32)

    exe = _build()
    zs = [exe["zmaker"]() for _ in range(NSTAGE)]   # async, on-device zeros

    # per-feature scale from a 1/16 row subsample (+5% headroom); the
    # quantizer clips, so an under-estimate only costs a little extra
    # rounding error on the few clipped values
    ucol = np.maximum(
        np.abs(inputs_np[::16]).max(axis=0) * np.float32(1.05),
        np.float32(1e-30))
    # constants are cached on device across calls keyed by content; any
    # change in A/B/C/mean/input scales recomputes and re-uploads
    kh = hashlib.blake2b(
        A.tobytes() + B.tobytes() + C.tobytes() + mean.tobytes()
        + ucol.tobytes(), digest_size=16).hexdigest()
    if _CACHE.get("kon_key") != kh:
        kon = _make_kon(mean, A, B, C, ucol)
        kon0 = jax.device_put(kon, exe["devs"][0])
        _CACHE["kon_rep"] = jax.device_put(kon0, exe["sh_rep"])
        _CACHE["kon_key"] = kh
    kon_rep = _CACHE["kon_rep"]

    uinv = (np.float32(127.0) / ucol)[:, None]
    out_devs = []
    for s in range(NSTAGE):
        u_dev = _stream_u(inputs_np, uinv, s, exe)
        try:
            (od,) = exe["sharded"](u_dev, kon_rep, zs[s])
        except Exception:
            # one retry: a previously crashed process can leave the exec
            # unit wedged; the failed attempt resets it
            (od,) = exe["sharded"](u_dev, kon_rep, exe["zmaker"]())
        od.copy_to_host_async()     # D2H starts as soon as exec finishes
        out_devs.append(od)

    # host correction while results stream back: out row n-1 +=
    # (A^n z0) @ B.T.  ||A^n z0|| ~ 0.9^n, so 64 rows reach ~1e-3 of a
    # unit (well under the int8 quantization noise).
    HC = 64
    z0 = R @ (data[0] - mean[0])
    zc = z0
    corr = np.empty((HC, DZ), np.float32)
    for n in range(1, HC + 1):
        zc = A @ zc
        corr[n - 1] = B @ zc

    out = np.empty((T, DZ), np.float32)
    for s in range(NSTAGE):
        buf = np.asarray(out_devs[s])           # blocks on stage D2H
        scale = (buf[:, DZ:OW].copy().view(np.float32)
                 * np.float32(1.0 / 127.0))
        for i in range(NCORE):
            r0 = i * TLOC + s * TLOC_S
            np.multiply(buf[i * TLOC_S:(i + 1) * TLOC_S, 0:DZ],
                        scale[i * TLOC_S:(i + 1) * TLOC_S],
                        out=out[r0:r0 + TLOC_S])
    out[:HC] += corr
    return out
